# revision 17
# baseline (speedup 1.0000x reference)
"""Trainium2 Bass kernel for NoisyPQC (12-qubit noisy PQC expectation values).

Math restructure (validated vs reference in numpy):
  state index s = p*32 + f;  p = qubits 0..6 (qubit 0 = MSB of p),
  f = qubits 7..11.  state.reshape(128,32)[p,f] == state[s].
  Per trajectory r:  x = B3 D B2 D B1 D B0 psi0 with
    B0 = U0, Bl = Ul*Perm(m_{l-1})*Diag(sigma_{l-1}),
    D = (D_P (x) D_F) * C,  C[p,f] = (-1)^((p&1)*((f>>4)&1)).
  D_P/D_F fold into columns of B_l (l>=1); C applied elementwise 3x.
  Layer 0 is trajectory-independent -> host precomputes
    state1 = C * (GP0 @ psi0 @ GF0^T)  for all 16 batches.
  Device per (r): 3 layers of  phi = GP_l @ phi @ GF_l^T  (x C for l<3),
  then probs = |phi|^2, contracted with z-sign tables; final-layer noise
  becomes sign flips folded into the measurement matrices.

v4 scheme ("stationary-state" matmuls as v2, plus):
  - gate tables store [-B|A|B] (1152 cols/rep/tensor): the two moving
    pair planes [A|B] and [-B|A] are overlapping 256-col windows, so the
    HBM gate stream drops 25%.
  - every gate stage is TWO half-stages of 512 psum cols (one bank);
    per-chain mm tag with bufs=2 decouples the PE from the PSUM->SBUF
    moves: the PE runs ahead into the other bank while a move drains.
  - THREE interleaved chains (reps 1..24) + rep 0 solo at the head where
    the PE is DMA-bound anyway.  Each chain's meas1/transpose PSUM lives
    inside its own mm-bank rotation -> 7 banks total, no cross-chain
    PSUM contention.
  - moves: h0 copy on ACT / h1 copy on DVE; xC multiplies on DVE (h0+h1
    via the C table); final-layer moves ARE the ACT squares (PSUM->SBUF
    with Square), no separate square pass, contiguous (not ri-major) -
    meas1 instead sums ri via two strided-moving matmuls per half.

Layouts (b = 4*b_hi + b_lo):
  N: [p=128 part, col = b_hi*256 + ri*128 + b_lo*32 + f]   (ri: 0=Re,1=Im)
  T: [b_lo*32+f = 128 part, col = b_hi*256 + ri*128 + p]
P-stage (N->T), per b_hi: mm pairA=[GPr^T|GPi^T] then pairB=[-GPi^T|GPr^T]
accumulated.  F-stage (T->N) same with kron(I4, .) planes.  All mms
stream 256 cols => full-speed f32r (1 cyc/row).

Sharding: 200 trajectories = 8 cores x 25. Each core accumulates its 25
trajectories' (sign-flipped) measurement vectors into one PSUM bank via a
single open accumulation group; host sums the 8 [24,32] outputs and /200.
"""

import sys

for _p in ("/opt/trn_rl_repo",):
    if _p not in sys.path:
        sys.path.insert(0, _p)

import numpy as np

from concourse import bacc, bass_utils, mybir
import concourse.tile as tile

# ---------------- problem constants (hardcoded per contract) ----------------
NQ = 12
NL = 4
REPS = 200
BATCH = 16
NCORES = 8
RL = REPS // NCORES  # 25 reps per core
DP, DF = 128, 32  # dim of p-side (qubits 0..6) and f-side (qubits 7..11)
GW = 384  # gate cols per layer: [-B|A|B]; pairB = cols 0:256, pairA = 128:384

F32 = mybir.dt.float32
F32R = mybir.dt.float32r


# ---------------- host-side math ----------------
def _rx(t):
    c, s = np.cos(0.5 * t), -1j * np.sin(0.5 * t)
    return np.array([[c, s], [s, c]], np.complex64)


def _rz(t):
    return np.array([[np.exp(-0.5j * t), 0], [0, np.exp(0.5j * t)]], np.complex64)


def _kron_list(mats):
    out = mats[0]
    for m in mats[1:]:
        out = np.kron(out, m)
    return out


def _tables():
    p = np.arange(DP)
    f = np.arange(DF)
    dP = np.ones(DP)
    for j in range(6):
        dP *= np.where(((p >> (6 - j)) & 1) & ((p >> (5 - j)) & 1), -1.0, 1.0)
    dF = np.ones(DF)
    for k in range(4):
        dF *= np.where(((f >> (4 - k)) & 1) & ((f >> (3 - k)) & 1), -1.0, 1.0)
    C = np.where(((p[:, None] & 1) & ((f[None, :] >> 4) & 1)) == 1, -1.0, 1.0)
    zP = 1.0 - 2.0 * ((p[None, :] >> (6 - np.arange(7)[:, None])) & 1)  # [7,128]
    zF = 1.0 - 2.0 * ((f[None, :] >> (4 - np.arange(5)[:, None])) & 1)  # [5,32]
    return dP, dF, C, zP, zF


def host_prep(data_angles, params, noise):
    """Build all device arrays. Returns (shared dict, per-core list of dicts)."""
    da = np.asarray(data_angles, np.float64)
    pa = np.asarray(params, np.float64)
    nz = np.asarray(noise)
    dPt, dFt, C, zP, zF = _tables()

    # base per-qubit gates u[l][q] = Rx(params[l,q,1]) @ Rz(params[l,q,0])
    u = [[_rx(pa[l, q, 1]) @ _rz(pa[l, q, 0]) for q in range(NQ)] for l in range(NL)]

    # --- state after layer 0 (incl. C), identical for every trajectory ---
    va = np.stack([np.cos(0.5 * da), -1j * np.sin(0.5 * da)], -1)  # [B,12,2]
    GP0 = _kron_list([u[0][q] for q in range(7)])        # [128,128]
    GF0 = _kron_list([u[0][q] for q in range(7, NQ)])    # [32,32]
    s_re = np.empty((DP, BATCH * DF), np.float32)
    s_im = np.empty((DP, BATCH * DF), np.float32)
    for b in range(BATCH):
        vPr = _kron_list([va[b, q].astype(np.complex128) for q in range(7)])
        vFr = _kron_list([va[b, q].astype(np.complex128) for q in range(7, NQ)])
        phi = (GP0 @ np.outer(vPr, vFr) @ GF0.T) * C
        s_re[:, b * DF:(b + 1) * DF] = phi.real
        s_im[:, b * DF:(b + 1) * DF] = phi.imag
    # N layout: col = b_hi*256 + ri*128 + b_lo*32 + f
    state1 = np.empty((DP, 1024), np.float32)
    for bh in range(4):
        state1[:, bh * 256:bh * 256 + 128] = s_re[:, bh * 128:bh * 128 + 128]
        state1[:, bh * 256 + 128:bh * 256 + 256] = s_im[:, bh * 128:bh * 128 + 128]

    # --- per-core moving gate tables + measurement tables ---
    # per layer block (GW=384 cols): [-B | A | B] with A = Re-plane, B =
    # Im-plane; pairA=[A|B] = cols 128:384, pairB=[-B|A] = cols 0:256.
    eye4 = np.eye(4)
    percore = []
    for c in range(NCORES):
        gp = np.empty((RL, DP, 3 * GW), np.float32)
        gf = np.empty((RL, 32, 9 * DF), np.float32)
        m1 = np.empty((DP, RL * 24), np.float32)
        m2 = np.empty((DP, RL * 8), np.float32)
        for rl in range(RL):
            r = c * RL + rl
            for lidx, l in enumerate((1, 2, 3)):
                facs = []
                for q in range(NQ):
                    g = u[l][q]
                    ch = int(nz[r, l - 1, q])
                    if ch in (1, 2):
                        g = g[:, ::-1]
                    if ch in (2, 3):
                        g = g * np.array([1.0, -1.0])[None, :]
                    facs.append(g)
                GP = _kron_list(facs[:7]) * dPt[None, :]
                GF = _kron_list(facs[7:]) * dFt[None, :]
                o = lidx * GW
                gp[rl, :, o:o + 128] = -GP.imag.T
                gp[rl, :, o + 128:o + 256] = GP.real.T
                gp[rl, :, o + 256:o + 384] = GP.imag.T
                of = lidx * 3 * DF
                gf[rl, :, of:of + DF] = -GF.imag.T
                gf[rl, :, of + DF:of + 2 * DF] = GF.real.T
                gf[rl, :, of + 2 * DF:of + 3 * DF] = GF.imag.T
            m3 = nz[r, NL - 1]  # [12]
            flip = np.where((m3 == 1) | (m3 == 2), -1.0, 1.0)
            M1 = np.ones((DF, 6))
            M1[:, :5] = (zF * flip[7:, None]).T  # [32,5]
            m1[:, rl * 24:(rl + 1) * 24] = np.kron(eye4, M1)
            M2 = np.ones((DP, 8), np.float64)
            M2[:, :7] = (zP * flip[:7, None]).T
            m2[:, rl * 8:(rl + 1) * 8] = M2
        percore.append({"gp": gp, "gf": gf, "meas1": m1, "meas2": m2})

    shared = {
        "state1": np.ascontiguousarray(state1),
        "ctabN": np.ascontiguousarray(np.tile(C, (1, 32)).astype(np.float32)),
        "ident": np.eye(8, dtype=np.float32),
    }
    for d in percore:
        for k in list(d):
            d[k] = np.ascontiguousarray(d[k], np.float32)
    return shared, percore


def decode_output(acc):
    """acc: [24,32] summed over cores. Returns [16,12] float32."""
    out = np.empty((BATCH, NQ), np.float64)
    for bh in range(4):
        for bl in range(4):
            b = 4 * bh + bl
            for q in range(7):
                out[b, q] = acc[bl * 6 + 5, bh * 8 + q]
            for j in range(5):
                out[b, 7 + j] = acc[bl * 6 + j, bh * 8 + 7]
    return (out / REPS).astype(np.float32)


# ---------------- device kernel ----------------
def build_nc(dt=F32R, debug=False):
    """Build + compile the per-core Bass program (same for all cores)."""
    nc = bacc.Bacc("TRN2", target_bir_lowering=False, debug=debug,
                   num_devices=NCORES)
    # float32r is bit-identical to float32; declaring the DRAM side as the
    # same dtype as the SBUF tiles keeps the (hardware-DGE) DMAs cast-free.
    d_state1 = nc.dram_tensor("state1", (DP, 1024), dt, kind="ExternalInput")
    d_gp = nc.dram_tensor("gp", (RL, DP, 3 * GW), dt, kind="ExternalInput")
    d_gf = nc.dram_tensor("gf", (RL, 32, 9 * DF), dt, kind="ExternalInput")
    d_m1 = nc.dram_tensor("meas1", (DP, RL * 24), dt, kind="ExternalInput")
    d_m2 = nc.dram_tensor("meas2", (DP, RL * 8), dt, kind="ExternalInput")
    d_ctn = nc.dram_tensor("ctabN", (DP, 1024), F32, kind="ExternalInput")
    d_id = nc.dram_tensor("ident", (8, 8), dt, kind="ExternalInput")
    d_out = nc.dram_tensor("out", (24, 32), F32, kind="ExternalOutput")

    MUL = mybir.AluOpType.mult
    SQ = mybir.ActivationFunctionType.Square

    with tile.TileContext(nc) as tc:
        from contextlib import ExitStack
        with ExitStack() as ex:
            cp = ex.enter_context(tc.tile_pool(name="const", bufs=1))
            sp = ex.enter_context(tc.tile_pool(name="work", bufs=1))
            pp = ex.enter_context(tc.tile_pool(name="ps", bufs=1, space="PSUM"))

            # constants -> SBUF once
            c_state1 = cp.tile([DP, 1024], dt, name="state1", tag="state1")
            c_ctn = cp.tile([DP, 1024], F32, name="ctn", tag="ctn")
            c_id = cp.tile([8, 8], dt, name="ident", tag="ident")
            c_m1 = cp.tile([DP, RL * 24], dt, name="m1", tag="m1")
            c_m2 = cp.tile([DP, RL * 8], dt, name="m2", tag="m2")
            # consts ride on the ACT queue's DGE ring so the sync+gpsimd
            # rings are free for the per-rep gate streams; need-order.
            nc.scalar.dma_start(c_state1[:, 0:512], d_state1.ap()[:, 0:512])
            nc.scalar.dma_start(c_state1[:, 512:1024], d_state1.ap()[:, 512:1024])
            nc.scalar.dma_start(c_ctn, d_ctn.ap())
            nc.scalar.dma_start(c_m2, d_m2.ap())
            nc.scalar.dma_start(c_m1, d_m1.ap())
            nc.scalar.dma_start(c_id, d_id.ap())

            macc = pp.tile([24, 32], F32, name="macc", tag="macc")
            macc_n = [0]

            # zero the gf double-buffers once: per-rep DMAs only write the
            # 4 diagonal kron blocks.  Two dummy allocations per tag walk
            # each rotation exactly one full cycle.
            for zch in range(3):
                for _ in range(2):
                    z = sp.tile([DP, 3 * GW], dt, name="gfz", tag="gf" + str(zch),
                                bufs=2)
                    nc.vector.memset(z.bitcast(mybir.dt.uint32), 0)

            def emit_rep(r, ch):
                """Return list of 16 stage closures for trajectory r, chain ch."""
                t = {}
                g = f"{ch}"

                def s_dma():
                    t["gp"] = sp.tile([DP, 3 * GW], dt, name="gp", tag="gp" + g, bufs=2)
                    t["gf"] = sp.tile([DP, 3 * GW], dt, name="gf", tag="gf" + g, bufs=2)
                    for l3 in range(3):
                        cs = slice(l3 * GW, (l3 + 1) * GW)
                        eng = nc.sync if l3 < 2 else nc.gpsimd
                        eng.dma_start(t["gp"][:, cs], d_gp.ap()[r][:, cs])
                    # gf planes are kron(I4, .): DMA only the 32-row content
                    # into the 4 diagonal blocks (off-diagonal zeros are
                    # pre-set once at startup and never touched).
                    gft = t["gf"]
                    srf = d_gf.ap()[r]
                    APc, VP = type(gft), type(gft.ap)
                    for bl in range(4):
                        dst = APc(tensor=gft.tensor,
                                  offset=gft.offset + bl * 32 * 1152 + bl * 32,
                                  ap=VP([[1152, 32], [128, 9], [1, 32]]))
                        srcb = APc(tensor=srf.tensor, offset=srf.offset,
                                   ap=VP([[288, 32], [32, 9], [1, 32]]))
                        nc.gpsimd.dma_start(dst, srcb)

                def new_mm():
                    return pp.tile([DP, 512], F32, name="mm", tag="mm" + g, bufs=2)

                def mk_half(stat_key, side, lidx, half, move):
                    """One half-stage: 4 matmuls (b_hi pair 2*half, 2*half+1)
                    into a single-bank [128,512] psum tile, then this half's
                    PSUM->SBUF move.  Stationary = state cols of the global
                    b_hi block; moving = the 256-col gate pair windows."""
                    def s():
                        mm = new_mm()
                        stat = c_state1 if stat_key is None else t[stat_key]
                        mov = t["gp"] if side == "P" else t["gf"]
                        base = lidx * GW
                        movA = mov[:, base + 128:base + 384]
                        movB = mov[:, base:base + 256]
                        for j, bh in enumerate((2 * half, 2 * half + 1)):
                            o, so = j * 256, bh * 256
                            nc.tensor.matmul(mm[:, o:o + 256],
                                             stat[:, so:so + 128], movA,
                                             start=True, stop=False)
                            nc.tensor.matmul(mm[:, o:o + 256],
                                             stat[:, so + 128:so + 256], movB,
                                             start=False, stop=True)
                        move(mm, half)
                    return s

                def mv_copy(key):
                    # h0 on ACT, h1 on DVE: each a single 512-col move with a
                    # multi-half-stage window before the bank is needed again.
                    def m(mm, half):
                        if half == 0:
                            t[key] = sp.tile([DP, 1024], dt, name="tmp",
                                             tag="tmp" + g, bufs=2)
                            nc.scalar.copy(t[key][:, 0:512], mm)
                        else:
                            nc.vector.tensor_copy(t[key][:, 512:1024], mm)
                    return m

                def mv_cmul(key):
                    # x C on DVE via the C table (ACT cannot tensor_tensor)
                    def m(mm, half):
                        if half == 0:
                            t[key] = sp.tile([DP, 1024], dt, name="st",
                                             tag="st" + g, bufs=2)
                        cs = slice(half * 512, (half + 1) * 512)
                        nc.vector.tensor_tensor(t[key][:, cs], mm, c_ctn[:, cs], MUL)
                    return m

                def mv_square(mm, half):
                    # final layer: the move IS the square (contiguous layout,
                    # cols (bh, ri, b_lo, f) for bh pair of this half)
                    key = "sqA" if half == 0 else "sqB"
                    t[key] = sp.tile([DP, 512], dt, name=key, tag=key + g, bufs=2)
                    nc.scalar.activation(t[key], mm, SQ)

                def s_meas1():
                    # m1p rides the chain's own mm-bank rotation: write into
                    # partitions 0:8 of a full-bank allocation.
                    mp = new_mm()
                    m1p = mp[0:8, :]
                    lhs = c_m2[:, r * 8:(r + 1) * 8]
                    for half in range(2):
                        sq = t["sqA"] if half == 0 else t["sqB"]
                        APc, VP = type(sq), type(sq.ap)
                        for ri in range(2):
                            mov = APc(tensor=sq.tensor,
                                      offset=sq.offset + ri * 128,
                                      ap=VP([[512, DP], [256, 2], [1, 128]]))
                            nc.tensor.matmul(m1p[:, half * 256:(half + 1) * 256],
                                             lhs, mov,
                                             start=(ri == 0), stop=(ri == 1))
                    t["m1s"] = sp.tile([8, 512], dt, name="m1s", tag="m1s" + g,
                                       bufs=2)
                    nc.vector.tensor_copy(t["m1s"], m1p)

                def s_meas_tr():
                    # transpose 4 [8,128] blocks -> trm [128, (bh,q)=32];
                    # trm also lives in the chain's mm-bank rotation.
                    trt = new_mm()
                    trm = trt[:, 0:32].bitcast(dt)
                    for bh in range(4):
                        nc.tensor.matmul(trm[:, bh * 8:bh * 8 + 8],
                                         t["m1s"][0:8, bh * 128:(bh + 1) * 128],
                                         c_id, is_transpose=True,
                                         start=(bh == 0), stop=(bh == 3))
                    t["trs"] = sp.tile([DP, 32], dt, name="trs", tag="trs" + g,
                                       bufs=2)
                    nc.vector.tensor_copy(t["trs"], trm)

                def s_macc():
                    macc_n[0] += 1
                    nc.tensor.matmul(macc, c_m1[:, r * 24:(r + 1) * 24], t["trs"],
                                     start=(macc_n[0] == 1),
                                     stop=(macc_n[0] == RL))

                st = [s_dma]
                plan = [(None, "P", 0, mv_copy("t1")),
                        ("t1", "F", 0, mv_cmul("s1")),
                        ("s1", "P", 1, mv_copy("t2")),
                        ("t2", "F", 1, mv_cmul("s2")),
                        ("s2", "P", 2, mv_copy("t3")),
                        ("t3", "F", 2, mv_square)]
                for stat_key, side, lidx, move in plan:
                    st.append(mk_half(stat_key, side, lidx, 0, move))
                    st.append(mk_half(stat_key, side, lidx, 1, move))
                st += [s_meas1, s_meas_tr, s_macc]
                return st

            # rep 0 runs SOLO at the head (the PE is DMA-bound there anyway);
            # then three chains cover reps 1..24, round-robin one stage at a
            # time so every engine sees a steady interleave.
            chains = [
                [emit_rep(rr, ch) for rr in range(1 + ch, RL, 3)]
                for ch in range(3)
            ]
            solo = emit_rep(0, 0)
            for s in solo:
                s()
            queues = [[s for rep in chain for s in rep] for chain in chains]
            pos = [0, 0, 0]
            while any(pos[i] < len(queues[i]) for i in range(3)):
                for i in range(3):
                    if pos[i] < len(queues[i]):
                        queues[i][pos[i]]()
                        pos[i] += 1

            # final: copy accumulator to SBUF, DMA out
            outs = sp.tile([24, 32], F32, name="outs", tag="outs")
            nc.vector.tensor_copy(outs, macc)
            nc.sync.dma_start(d_out.ap(), outs)

    nc.compile()
    return nc


# ---------------- public entry ----------------
_CACHE = {}


def _get_nc():
    if "nc" not in _CACHE:
        _CACHE["nc"] = build_nc()
    return _CACHE["nc"]


def run(inputs, trace=False):
    shared, percore = host_prep(inputs["data_angles"], inputs["params"],
                                inputs["noise_choices"])
    nc = _get_nc()
    in_maps = []
    for c in range(NCORES):
        m = dict(shared)
        m.update(percore[c])
        in_maps.append(m)
    res = bass_utils.run_bass_kernel_spmd(nc, in_maps, list(range(NCORES)),
                                          trace=trace)
    acc = np.zeros((24, 32), np.float64)
    for c in range(NCORES):
        acc += np.asarray(res.results[c]["out"], np.float64)
    return decode_output(acc), res


def kernel(**inputs):
    out, _ = run(inputs)
    return out


# revision 18
# speedup vs baseline: 1.0317x; 1.0317x over previous
"""Trainium2 Bass kernel for NoisyPQC (12-qubit noisy PQC expectation values).

Math restructure (validated vs reference in numpy):
  state index s = p*32 + f;  p = qubits 0..6 (qubit 0 = MSB of p),
  f = qubits 7..11.  state.reshape(128,32)[p,f] == state[s].
  Per trajectory r:  x = B3 D B2 D B1 D B0 psi0 with
    B0 = U0, Bl = Ul*Perm(m_{l-1})*Diag(sigma_{l-1}),
    D = (D_P (x) D_F) * C,  C[p,f] = (-1)^((p&1)*((f>>4)&1)).
  D_P/D_F fold into columns of B_l (l>=1); C applied elementwise 3x.
  Layer 0 is trajectory-independent -> host precomputes
    state1 = C * (GP0 @ psi0 @ GF0^T)  for all 16 batches.
  Device per (r): 3 layers of  phi = GP_l @ phi @ GF_l^T  (x C for l<3),
  then probs = |phi|^2, contracted with z-sign tables; final-layer noise
  becomes sign flips folded into the measurement matrices.

v4 scheme ("stationary-state" matmuls as v2, plus):
  - gate tables store [-B|A|B] (1152 cols/rep/tensor): the two moving
    pair planes [A|B] and [-B|A] are overlapping 256-col windows, so the
    HBM gate stream drops 25%.
  - every gate stage is TWO half-stages of 512 psum cols (one bank);
    per-chain mm tag with bufs=2 decouples the PE from the PSUM->SBUF
    moves: the PE runs ahead into the other bank while a move drains.
  - THREE interleaved chains (reps 1..24) + rep 0 solo at the head where
    the PE is DMA-bound anyway.  Each chain's meas1/transpose PSUM lives
    inside its own mm-bank rotation -> 7 banks total, no cross-chain
    PSUM contention.
  - moves: h0 copy on ACT / h1 copy on DVE; xC multiplies on DVE (h0+h1
    via the C table); final-layer moves ARE the ACT squares (PSUM->SBUF
    with Square), no separate square pass, contiguous (not ri-major) -
    meas1 instead sums ri via two strided-moving matmuls per half.

Layouts (b = 4*b_hi + b_lo):
  N: [p=128 part, col = b_hi*256 + ri*128 + b_lo*32 + f]   (ri: 0=Re,1=Im)
  T: [b_lo*32+f = 128 part, col = b_hi*256 + ri*128 + p]
P-stage (N->T), per b_hi: mm pairA=[GPr^T|GPi^T] then pairB=[-GPi^T|GPr^T]
accumulated.  F-stage (T->N) same with kron(I4, .) planes.  All mms
stream 256 cols => full-speed f32r (1 cyc/row).

Sharding: 200 trajectories = 8 cores x 25. Each core accumulates its 25
trajectories' (sign-flipped) measurement vectors into one PSUM bank via a
single open accumulation group; host sums the 8 [24,32] outputs and /200.
"""

import sys

for _p in ("/opt/trn_rl_repo",):
    if _p not in sys.path:
        sys.path.insert(0, _p)

import numpy as np

from concourse import bacc, bass_utils, mybir
import concourse.tile as tile

# ---------------- problem constants (hardcoded per contract) ----------------
# reps whose gf tile is that physical buffer's FIRST use (full-plane DMA,
# which also seeds the off-diagonal zeros); all later reps only scatter the
# 4 diagonal kron blocks.
_FULL_GF_REPS = [0, 1, 2, 3, 5, 6]
NQ = 12
NL = 4
REPS = 200
BATCH = 16
NCORES = 8
RL = REPS // NCORES  # 25 reps per core
DP, DF = 128, 32  # dim of p-side (qubits 0..6) and f-side (qubits 7..11)
GW = 384  # gate cols per layer: [-B|A|B]; pairB = cols 0:256, pairA = 128:384

F32 = mybir.dt.float32
F32R = mybir.dt.float32r


# ---------------- host-side math ----------------
def _rx(t):
    c, s = np.cos(0.5 * t), -1j * np.sin(0.5 * t)
    return np.array([[c, s], [s, c]], np.complex64)


def _rz(t):
    return np.array([[np.exp(-0.5j * t), 0], [0, np.exp(0.5j * t)]], np.complex64)


def _kron_list(mats):
    out = mats[0]
    for m in mats[1:]:
        out = np.kron(out, m)
    return out


def _tables():
    p = np.arange(DP)
    f = np.arange(DF)
    dP = np.ones(DP)
    for j in range(6):
        dP *= np.where(((p >> (6 - j)) & 1) & ((p >> (5 - j)) & 1), -1.0, 1.0)
    dF = np.ones(DF)
    for k in range(4):
        dF *= np.where(((f >> (4 - k)) & 1) & ((f >> (3 - k)) & 1), -1.0, 1.0)
    C = np.where(((p[:, None] & 1) & ((f[None, :] >> 4) & 1)) == 1, -1.0, 1.0)
    zP = 1.0 - 2.0 * ((p[None, :] >> (6 - np.arange(7)[:, None])) & 1)  # [7,128]
    zF = 1.0 - 2.0 * ((f[None, :] >> (4 - np.arange(5)[:, None])) & 1)  # [5,32]
    return dP, dF, C, zP, zF


def host_prep(data_angles, params, noise):
    """Build all device arrays. Returns (shared dict, per-core list of dicts)."""
    da = np.asarray(data_angles, np.float64)
    pa = np.asarray(params, np.float64)
    nz = np.asarray(noise)
    dPt, dFt, C, zP, zF = _tables()

    # base per-qubit gates u[l][q] = Rx(params[l,q,1]) @ Rz(params[l,q,0])
    u = [[_rx(pa[l, q, 1]) @ _rz(pa[l, q, 0]) for q in range(NQ)] for l in range(NL)]

    # --- state after layer 0 (incl. C), identical for every trajectory ---
    va = np.stack([np.cos(0.5 * da), -1j * np.sin(0.5 * da)], -1)  # [B,12,2]
    GP0 = _kron_list([u[0][q] for q in range(7)])        # [128,128]
    GF0 = _kron_list([u[0][q] for q in range(7, NQ)])    # [32,32]
    s_re = np.empty((DP, BATCH * DF), np.float32)
    s_im = np.empty((DP, BATCH * DF), np.float32)
    for b in range(BATCH):
        vPr = _kron_list([va[b, q].astype(np.complex128) for q in range(7)])
        vFr = _kron_list([va[b, q].astype(np.complex128) for q in range(7, NQ)])
        phi = (GP0 @ np.outer(vPr, vFr) @ GF0.T) * C
        s_re[:, b * DF:(b + 1) * DF] = phi.real
        s_im[:, b * DF:(b + 1) * DF] = phi.imag
    # N layout: col = b_hi*256 + ri*128 + b_lo*32 + f
    state1 = np.empty((DP, 1024), np.float32)
    for bh in range(4):
        state1[:, bh * 256:bh * 256 + 128] = s_re[:, bh * 128:bh * 128 + 128]
        state1[:, bh * 256 + 128:bh * 256 + 256] = s_im[:, bh * 128:bh * 128 + 128]

    # --- per-core moving gate tables + measurement tables ---
    # per layer block (GW=384 cols): [-B | A | B] with A = Re-plane, B =
    # Im-plane; pairA=[A|B] = cols 128:384, pairB=[-B|A] = cols 0:256.
    eye4 = np.eye(4)
    percore = []
    for c in range(NCORES):
        gp = np.empty((RL, DP, 3 * GW), np.float32)
        gf = np.empty((RL, 32, 9 * DF), np.float32)
        gff = np.zeros((6, DP, 3 * GW), np.float32)  # full planes, first-buffer reps
        m1 = np.empty((DP, RL * 24), np.float32)
        m2 = np.empty((DP, RL * 8), np.float32)
        for rl in range(RL):
            r = c * RL + rl
            for lidx, l in enumerate((1, 2, 3)):
                facs = []
                for q in range(NQ):
                    g = u[l][q]
                    ch = int(nz[r, l - 1, q])
                    if ch in (1, 2):
                        g = g[:, ::-1]
                    if ch in (2, 3):
                        g = g * np.array([1.0, -1.0])[None, :]
                    facs.append(g)
                GP = _kron_list(facs[:7]) * dPt[None, :]
                GF = _kron_list(facs[7:]) * dFt[None, :]
                o = lidx * GW
                gp[rl, :, o:o + 128] = -GP.imag.T
                gp[rl, :, o + 128:o + 256] = GP.real.T
                gp[rl, :, o + 256:o + 384] = GP.imag.T
                of = lidx * 3 * DF
                gf[rl, :, of:of + DF] = -GF.imag.T
                gf[rl, :, of + DF:of + 2 * DF] = GF.real.T
                gf[rl, :, of + 2 * DF:of + 3 * DF] = GF.imag.T
                if rl in _FULL_GF_REPS:
                    fi = _FULL_GF_REPS.index(rl)
                    A = np.kron(eye4, GF.real.T)
                    Bm = np.kron(eye4, GF.imag.T)
                    gff[fi, :, o:o + 128] = -Bm
                    gff[fi, :, o + 128:o + 256] = A
                    gff[fi, :, o + 256:o + 384] = Bm
            m3 = nz[r, NL - 1]  # [12]
            flip = np.where((m3 == 1) | (m3 == 2), -1.0, 1.0)
            M1 = np.ones((DF, 6))
            M1[:, :5] = (zF * flip[7:, None]).T  # [32,5]
            m1[:, rl * 24:(rl + 1) * 24] = np.kron(eye4, M1)
            M2 = np.ones((DP, 8), np.float64)
            M2[:, :7] = (zP * flip[:7, None]).T
            m2[:, rl * 8:(rl + 1) * 8] = M2
        percore.append({"gp": gp, "gf": gf, "gffull": gff, "meas1": m1, "meas2": m2})

    shared = {
        "state1": np.ascontiguousarray(state1),
        "ctabN": np.ascontiguousarray(np.tile(C, (1, 32)).astype(np.float32)),
        "ident": np.eye(8, dtype=np.float32),
    }
    for d in percore:
        for k in list(d):
            d[k] = np.ascontiguousarray(d[k], np.float32)
    return shared, percore


def decode_output(acc):
    """acc: [24,32] summed over cores. Returns [16,12] float32."""
    out = np.empty((BATCH, NQ), np.float64)
    for bh in range(4):
        for bl in range(4):
            b = 4 * bh + bl
            for q in range(7):
                out[b, q] = acc[bl * 6 + 5, bh * 8 + q]
            for j in range(5):
                out[b, 7 + j] = acc[bl * 6 + j, bh * 8 + 7]
    return (out / REPS).astype(np.float32)


# ---------------- device kernel ----------------
def build_nc(dt=F32R, debug=False):
    """Build + compile the per-core Bass program (same for all cores)."""
    nc = bacc.Bacc("TRN2", target_bir_lowering=False, debug=debug,
                   num_devices=NCORES)
    # float32r is bit-identical to float32; declaring the DRAM side as the
    # same dtype as the SBUF tiles keeps the (hardware-DGE) DMAs cast-free.
    d_state1 = nc.dram_tensor("state1", (DP, 1024), dt, kind="ExternalInput")
    d_gp = nc.dram_tensor("gp", (RL, DP, 3 * GW), dt, kind="ExternalInput")
    d_gf = nc.dram_tensor("gf", (RL, 32, 9 * DF), dt, kind="ExternalInput")
    d_gff = nc.dram_tensor("gffull", (6, DP, 3 * GW), dt, kind="ExternalInput")
    d_m1 = nc.dram_tensor("meas1", (DP, RL * 24), dt, kind="ExternalInput")
    d_m2 = nc.dram_tensor("meas2", (DP, RL * 8), dt, kind="ExternalInput")
    d_ctn = nc.dram_tensor("ctabN", (DP, 1024), F32, kind="ExternalInput")
    d_id = nc.dram_tensor("ident", (8, 8), dt, kind="ExternalInput")
    d_out = nc.dram_tensor("out", (24, 32), F32, kind="ExternalOutput")

    MUL = mybir.AluOpType.mult
    SQ = mybir.ActivationFunctionType.Square

    with tile.TileContext(nc) as tc:
        from contextlib import ExitStack
        with ExitStack() as ex:
            cp = ex.enter_context(tc.tile_pool(name="const", bufs=1))
            sp = ex.enter_context(tc.tile_pool(name="work", bufs=1))
            pp = ex.enter_context(tc.tile_pool(name="ps", bufs=1, space="PSUM"))

            # constants -> SBUF once
            c_state1 = cp.tile([DP, 1024], dt, name="state1", tag="state1")
            c_ctn = cp.tile([DP, 1024], F32, name="ctn", tag="ctn")
            c_id = cp.tile([8, 8], dt, name="ident", tag="ident")
            c_m1 = cp.tile([DP, RL * 24], dt, name="m1", tag="m1")
            c_m2 = cp.tile([DP, RL * 8], dt, name="m2", tag="m2")
            # consts ride on the ACT queue's DGE ring so the sync+gpsimd
            # rings are free for the per-rep gate streams; need-order.
            nc.scalar.dma_start(c_state1[:, 0:512], d_state1.ap()[:, 0:512])
            nc.scalar.dma_start(c_state1[:, 512:1024], d_state1.ap()[:, 512:1024])
            nc.scalar.dma_start(c_ctn, d_ctn.ap())
            nc.scalar.dma_start(c_m2, d_m2.ap())
            nc.scalar.dma_start(c_m1, d_m1.ap())
            nc.scalar.dma_start(c_id, d_id.ap())

            macc = pp.tile([24, 32], F32, name="macc", tag="macc")
            macc_n = [0]

            def emit_rep(r, ch):
                """Return list of 16 stage closures for trajectory r, chain ch."""
                t = {}
                g = f"{ch}"

                def s_dma():
                    t["gp"] = sp.tile([DP, 3 * GW], dt, name="gp", tag="gp" + g, bufs=2)
                    t["gf"] = sp.tile([DP, 3 * GW], dt, name="gf", tag="gf" + g, bufs=2)
                    for l3 in range(3):
                        cs = slice(l3 * GW, (l3 + 1) * GW)
                        eng = nc.sync if l3 < 2 else nc.gpsimd
                        eng.dma_start(t["gp"][:, cs], d_gp.ap()[r][:, cs])
                    # gf planes are kron(I4, .).  The first use of each
                    # physical buffer streams the full planes (seeding the
                    # off-diagonal zeros); every later rep only scatters the
                    # 32-row content into the 4 diagonal blocks (25% of the
                    # bytes).
                    gft = t["gf"]
                    if r in _FULL_GF_REPS:
                        fi = _FULL_GF_REPS.index(r)
                        for l3 in range(3):
                            cs = slice(l3 * GW, (l3 + 1) * GW)
                            nc.gpsimd.dma_start(gft[:, cs], d_gff.ap()[fi][:, cs])
                    else:
                        srf = d_gf.ap()[r]
                        APc, VP = type(gft), type(gft.ap)
                        for bl in range(4):
                            dst = APc(tensor=gft.tensor,
                                      offset=gft.offset + bl * 32 * 1152 + bl * 32,
                                      ap=VP([[1152, 32], [128, 9], [1, 32]]))
                            srcb = APc(tensor=srf.tensor, offset=srf.offset,
                                       ap=VP([[288, 32], [32, 9], [1, 32]]))
                            nc.gpsimd.dma_start(dst, srcb)

                def new_mm():
                    return pp.tile([DP, 512], F32, name="mm", tag="mm" + g, bufs=2)

                def mk_half(stat_key, side, lidx, half, move):
                    """One half-stage: 4 matmuls (b_hi pair 2*half, 2*half+1)
                    into a single-bank [128,512] psum tile, then this half's
                    PSUM->SBUF move.  Stationary = state cols of the global
                    b_hi block; moving = the 256-col gate pair windows."""
                    def s():
                        mm = new_mm()
                        stat = c_state1 if stat_key is None else t[stat_key]
                        mov = t["gp"] if side == "P" else t["gf"]
                        base = lidx * GW
                        movA = mov[:, base + 128:base + 384]
                        movB = mov[:, base:base + 256]
                        for j, bh in enumerate((2 * half, 2 * half + 1)):
                            o, so = j * 256, bh * 256
                            nc.tensor.matmul(mm[:, o:o + 256],
                                             stat[:, so:so + 128], movA,
                                             start=True, stop=False)
                            nc.tensor.matmul(mm[:, o:o + 256],
                                             stat[:, so + 128:so + 256], movB,
                                             start=False, stop=True)
                        move(mm, half)
                    return s

                def mv_copy(key):
                    # h0 on ACT, h1 on DVE: each a single 512-col move with a
                    # multi-half-stage window before the bank is needed again.
                    def m(mm, half):
                        if half == 0:
                            t[key] = sp.tile([DP, 1024], dt, name="tmp",
                                             tag="tmp" + g, bufs=2)
                            nc.scalar.copy(t[key][:, 0:512], mm)
                        else:
                            nc.vector.tensor_copy(t[key][:, 512:1024], mm)
                    return m

                def mv_cmul(key):
                    # x C on DVE via the C table (ACT cannot tensor_tensor)
                    def m(mm, half):
                        if half == 0:
                            t[key] = sp.tile([DP, 1024], dt, name="st",
                                             tag="st" + g, bufs=2)
                        cs = slice(half * 512, (half + 1) * 512)
                        nc.vector.tensor_tensor(t[key][:, cs], mm, c_ctn[:, cs], MUL)
                    return m

                def mv_square(mm, half):
                    # final layer: the move IS the square (contiguous layout,
                    # cols (bh, ri, b_lo, f) for bh pair of this half)
                    key = "sqA" if half == 0 else "sqB"
                    t[key] = sp.tile([DP, 512], dt, name=key, tag=key + g, bufs=2)
                    nc.scalar.activation(t[key], mm, SQ)

                def s_meas1():
                    # m1p rides the chain's own mm-bank rotation: write into
                    # partitions 0:8 of a full-bank allocation.
                    mp = new_mm()
                    m1p = mp[0:8, :]
                    lhs = c_m2[:, r * 8:(r + 1) * 8]
                    for half in range(2):
                        sq = t["sqA"] if half == 0 else t["sqB"]
                        APc, VP = type(sq), type(sq.ap)
                        for ri in range(2):
                            mov = APc(tensor=sq.tensor,
                                      offset=sq.offset + ri * 128,
                                      ap=VP([[512, DP], [256, 2], [1, 128]]))
                            nc.tensor.matmul(m1p[:, half * 256:(half + 1) * 256],
                                             lhs, mov,
                                             start=(ri == 0), stop=(ri == 1))
                    t["m1s"] = sp.tile([8, 512], dt, name="m1s", tag="m1s" + g,
                                       bufs=2)
                    nc.vector.tensor_copy(t["m1s"], m1p)

                def s_meas_tr():
                    # transpose 4 [8,128] blocks -> trm [128, (bh,q)=32];
                    # trm also lives in the chain's mm-bank rotation.
                    trt = new_mm()
                    trm = trt[:, 0:32].bitcast(dt)
                    for bh in range(4):
                        nc.tensor.matmul(trm[:, bh * 8:bh * 8 + 8],
                                         t["m1s"][0:8, bh * 128:(bh + 1) * 128],
                                         c_id, is_transpose=True,
                                         start=(bh == 0), stop=(bh == 3))
                    t["trs"] = sp.tile([DP, 32], dt, name="trs", tag="trs" + g,
                                       bufs=2)
                    nc.vector.tensor_copy(t["trs"], trm)

                def s_macc():
                    macc_n[0] += 1
                    nc.tensor.matmul(macc, c_m1[:, r * 24:(r + 1) * 24], t["trs"],
                                     start=(macc_n[0] == 1),
                                     stop=(macc_n[0] == RL))

                st = [s_dma]
                plan = [(None, "P", 0, mv_copy("t1")),
                        ("t1", "F", 0, mv_cmul("s1")),
                        ("s1", "P", 1, mv_copy("t2")),
                        ("t2", "F", 1, mv_cmul("s2")),
                        ("s2", "P", 2, mv_copy("t3")),
                        ("t3", "F", 2, mv_square)]
                for stat_key, side, lidx, move in plan:
                    st.append(mk_half(stat_key, side, lidx, 0, move))
                    st.append(mk_half(stat_key, side, lidx, 1, move))
                st += [s_meas1, s_meas_tr, s_macc]
                return st

            # rep 0 runs SOLO at the head (the PE is DMA-bound there anyway);
            # then three chains cover reps 1..24, round-robin one stage at a
            # time so every engine sees a steady interleave.
            chains = [
                [emit_rep(rr, ch) for rr in range(1 + ch, RL, 3)]
                for ch in range(3)
            ]
            solo = emit_rep(0, 0)
            for s in solo:
                s()
            queues = [[s for rep in chain for s in rep] for chain in chains]
            pos = [0, 0, 0]
            while any(pos[i] < len(queues[i]) for i in range(3)):
                for i in range(3):
                    if pos[i] < len(queues[i]):
                        queues[i][pos[i]]()
                        pos[i] += 1

            # final: copy accumulator to SBUF, DMA out
            outs = sp.tile([24, 32], F32, name="outs", tag="outs")
            nc.vector.tensor_copy(outs, macc)
            nc.sync.dma_start(d_out.ap(), outs)

    nc.compile()
    return nc


# ---------------- public entry ----------------
_CACHE = {}


def _get_nc():
    if "nc" not in _CACHE:
        _CACHE["nc"] = build_nc()
    return _CACHE["nc"]


def run(inputs, trace=False):
    shared, percore = host_prep(inputs["data_angles"], inputs["params"],
                                inputs["noise_choices"])
    nc = _get_nc()
    in_maps = []
    for c in range(NCORES):
        m = dict(shared)
        m.update(percore[c])
        in_maps.append(m)
    res = bass_utils.run_bass_kernel_spmd(nc, in_maps, list(range(NCORES)),
                                          trace=trace)
    acc = np.zeros((24, 32), np.float64)
    for c in range(NCORES):
        acc += np.asarray(res.results[c]["out"], np.float64)
    return decode_output(acc), res


def kernel(**inputs):
    out, _ = run(inputs)
    return out


# revision 19
# speedup vs baseline: 1.1472x; 1.1120x over previous
"""Trainium2 Bass kernel for NoisyPQC (12-qubit noisy PQC expectation values).

Math restructure (validated vs reference in numpy):
  state index s = p*32 + f;  p = qubits 0..6 (qubit 0 = MSB of p),
  f = qubits 7..11.  state.reshape(128,32)[p,f] == state[s].
  Per trajectory r:  x = B3 D B2 D B1 D B0 psi0 with
    B0 = U0, Bl = Ul*Perm(m_{l-1})*Diag(sigma_{l-1}),
    D = (D_P (x) D_F) * C,  C[p,f] = (-1)^((p&1)*((f>>4)&1)).
  D_P/D_F fold into columns of B_l (l>=1); C applied elementwise 3x.
  Layer 0 is trajectory-independent -> host precomputes
    state1 = C * (GP0 @ psi0 @ GF0^T)  for all 16 batches.
  Device per (r): 3 layers of  phi = GP_l @ phi @ GF_l^T  (x C for l<3),
  then probs = |phi|^2, contracted with z-sign tables; final-layer noise
  becomes sign flips folded into the measurement matrices.

v4 scheme ("stationary-state" matmuls as v2, plus):
  - gate tables store [-B|A|B] (1152 cols/rep/tensor): the two moving
    pair planes [A|B] and [-B|A] are overlapping 256-col windows, so the
    HBM gate stream drops 25%.
  - every gate stage is TWO half-stages of 512 psum cols (one bank);
    per-chain mm tag with bufs=2 decouples the PE from the PSUM->SBUF
    moves: the PE runs ahead into the other bank while a move drains.
  - THREE interleaved chains (reps 1..24) + rep 0 solo at the head where
    the PE is DMA-bound anyway.  Each chain's meas1/transpose PSUM lives
    inside its own mm-bank rotation -> 7 banks total, no cross-chain
    PSUM contention.
  - moves: h0 copy on ACT / h1 copy on DVE; xC multiplies on DVE (h0+h1
    via the C table); final-layer moves ARE the ACT squares (PSUM->SBUF
    with Square), no separate square pass, contiguous (not ri-major) -
    meas1 instead sums ri via two strided-moving matmuls per half.

Layouts (b = 4*b_hi + b_lo):
  N: [p=128 part, col = b_hi*256 + ri*128 + b_lo*32 + f]   (ri: 0=Re,1=Im)
  T: [b_lo*32+f = 128 part, col = b_hi*256 + ri*128 + p]
P-stage (N->T), per b_hi: mm pairA=[GPr^T|GPi^T] then pairB=[-GPi^T|GPr^T]
accumulated.  F-stage (T->N) same with kron(I4, .) planes.  All mms
stream 256 cols => full-speed f32r (1 cyc/row).

Sharding: 200 trajectories = 8 cores x 25. Each core accumulates its 25
trajectories' (sign-flipped) measurement vectors into one PSUM bank via a
single open accumulation group; host sums the 8 [24,32] outputs and /200.
"""

import sys

for _p in ("/opt/trn_rl_repo",):
    if _p not in sys.path:
        sys.path.insert(0, _p)

import numpy as np

from concourse import bacc, bass_utils, mybir
import concourse.tile as tile

# ---------------- problem constants (hardcoded per contract) ----------------
# reps whose gf tile is that physical buffer's FIRST use (full-plane DMA,
# which also seeds the off-diagonal zeros); all later reps only scatter the
# 4 diagonal kron blocks.
_FULL_GF_REPS = [0, 1, 2, 3, 5, 6]
NQ = 12
NL = 4
REPS = 200
BATCH = 16
NCORES = 8
RL = REPS // NCORES  # 25 reps per core
DP, DF = 128, 32  # dim of p-side (qubits 0..6) and f-side (qubits 7..11)
GW = 384  # gate cols per layer: [-B|A|B]; pairB = cols 0:256, pairA = 128:384

F32 = mybir.dt.float32
F32R = mybir.dt.float32r
FP16 = mybir.dt.float16


# ---------------- host-side math ----------------
def _rx(t):
    c, s = np.cos(0.5 * t), -1j * np.sin(0.5 * t)
    return np.array([[c, s], [s, c]], np.complex64)


def _rz(t):
    return np.array([[np.exp(-0.5j * t), 0], [0, np.exp(0.5j * t)]], np.complex64)


def _kron_list(mats):
    out = mats[0]
    for m in mats[1:]:
        out = np.kron(out, m)
    return out


def _tables():
    p = np.arange(DP)
    f = np.arange(DF)
    dP = np.ones(DP)
    for j in range(6):
        dP *= np.where(((p >> (6 - j)) & 1) & ((p >> (5 - j)) & 1), -1.0, 1.0)
    dF = np.ones(DF)
    for k in range(4):
        dF *= np.where(((f >> (4 - k)) & 1) & ((f >> (3 - k)) & 1), -1.0, 1.0)
    C = np.where(((p[:, None] & 1) & ((f[None, :] >> 4) & 1)) == 1, -1.0, 1.0)
    zP = 1.0 - 2.0 * ((p[None, :] >> (6 - np.arange(7)[:, None])) & 1)  # [7,128]
    zF = 1.0 - 2.0 * ((f[None, :] >> (4 - np.arange(5)[:, None])) & 1)  # [5,32]
    return dP, dF, C, zP, zF


def host_prep(data_angles, params, noise):
    """Build all device arrays. Returns (shared dict, per-core list of dicts)."""
    da = np.asarray(data_angles, np.float64)
    pa = np.asarray(params, np.float64)
    nz = np.asarray(noise)
    dPt, dFt, C, zP, zF = _tables()

    # base per-qubit gates u[l][q] = Rx(params[l,q,1]) @ Rz(params[l,q,0])
    u = [[_rx(pa[l, q, 1]) @ _rz(pa[l, q, 0]) for q in range(NQ)] for l in range(NL)]

    # --- state after layer 0 (incl. C), identical for every trajectory ---
    va = np.stack([np.cos(0.5 * da), -1j * np.sin(0.5 * da)], -1)  # [B,12,2]
    GP0 = _kron_list([u[0][q] for q in range(7)])        # [128,128]
    GF0 = _kron_list([u[0][q] for q in range(7, NQ)])    # [32,32]
    s_re = np.empty((DP, BATCH * DF), np.float32)
    s_im = np.empty((DP, BATCH * DF), np.float32)
    for b in range(BATCH):
        vPr = _kron_list([va[b, q].astype(np.complex128) for q in range(7)])
        vFr = _kron_list([va[b, q].astype(np.complex128) for q in range(7, NQ)])
        phi = (GP0 @ np.outer(vPr, vFr) @ GF0.T) * C
        s_re[:, b * DF:(b + 1) * DF] = phi.real
        s_im[:, b * DF:(b + 1) * DF] = phi.imag
    # N layout: col = b_hi*256 + ri*128 + b_lo*32 + f
    state1 = np.empty((DP, 1024), np.float32)
    for bh in range(4):
        state1[:, bh * 256:bh * 256 + 128] = s_re[:, bh * 128:bh * 128 + 128]
        state1[:, bh * 256 + 128:bh * 256 + 256] = s_im[:, bh * 128:bh * 128 + 128]

    # --- per-core moving gate tables + measurement tables ---
    # per layer block (GW=384 cols): [-B | A | B] with A = Re-plane, B =
    # Im-plane; pairA=[A|B] = cols 128:384, pairB=[-B|A] = cols 0:256.
    eye4 = np.eye(4)
    percore = []
    for c in range(NCORES):
        gp = np.empty((RL, DP, 3 * GW), np.float32)
        gf = np.empty((RL, 32, 9 * DF), np.float32)
        gff = np.zeros((6, DP, 3 * GW), np.float32)  # full planes, first-buffer reps
        m1 = np.empty((DP, RL * 24), np.float32)
        m2 = np.empty((DP, RL * 8), np.float32)
        for rl in range(RL):
            r = c * RL + rl
            for lidx, l in enumerate((1, 2, 3)):
                facs = []
                for q in range(NQ):
                    g = u[l][q]
                    ch = int(nz[r, l - 1, q])
                    if ch in (1, 2):
                        g = g[:, ::-1]
                    if ch in (2, 3):
                        g = g * np.array([1.0, -1.0])[None, :]
                    facs.append(g)
                GP = _kron_list(facs[:7]) * dPt[None, :]
                GF = _kron_list(facs[7:]) * dFt[None, :]
                o = lidx * GW
                gp[rl, :, o:o + 128] = -GP.imag.T
                gp[rl, :, o + 128:o + 256] = GP.real.T
                gp[rl, :, o + 256:o + 384] = GP.imag.T
                of = lidx * 3 * DF
                gf[rl, :, of:of + DF] = -GF.imag.T
                gf[rl, :, of + DF:of + 2 * DF] = GF.real.T
                gf[rl, :, of + 2 * DF:of + 3 * DF] = GF.imag.T
                if rl in _FULL_GF_REPS:
                    fi = _FULL_GF_REPS.index(rl)
                    A = np.kron(eye4, GF.real.T)
                    Bm = np.kron(eye4, GF.imag.T)
                    gff[fi, :, o:o + 128] = -Bm
                    gff[fi, :, o + 128:o + 256] = A
                    gff[fi, :, o + 256:o + 384] = Bm
            m3 = nz[r, NL - 1]  # [12]
            flip = np.where((m3 == 1) | (m3 == 2), -1.0, 1.0)
            M1 = np.ones((DF, 6))
            M1[:, :5] = (zF * flip[7:, None]).T  # [32,5]
            m1[:, rl * 24:(rl + 1) * 24] = np.kron(eye4, M1)
            M2 = np.ones((DP, 8), np.float64)
            M2[:, :7] = (zP * flip[:7, None]).T
            m2[:, rl * 8:(rl + 1) * 8] = M2
        percore.append({"gp": gp, "gf": gf, "gffull": gff, "meas1": m1, "meas2": m2})

    shared = {
        "state1": np.ascontiguousarray(state1),
        "ctabN": np.ascontiguousarray(np.tile(C, (1, 32)).astype(np.float32)),
        "ident": np.eye(8, dtype=np.float32),
    }
    for d in percore:
        for k in list(d):
            d[k] = np.ascontiguousarray(d[k], np.float16)
    for k in ("state1", "ident"):
        shared[k] = np.ascontiguousarray(shared[k], np.float16)
    return shared, percore


def decode_output(acc):
    """acc: [24,32] summed over cores. Returns [16,12] float32."""
    out = np.empty((BATCH, NQ), np.float64)
    for bh in range(4):
        for bl in range(4):
            b = 4 * bh + bl
            for q in range(7):
                out[b, q] = acc[bl * 6 + 5, bh * 8 + q]
            for j in range(5):
                out[b, 7 + j] = acc[bl * 6 + j, bh * 8 + 7]
    return (out / REPS).astype(np.float32)


# ---------------- device kernel ----------------
def build_nc(dt=FP16, debug=False):
    """Build + compile the per-core Bass program (same for all cores)."""
    nc = bacc.Bacc("TRN2", target_bir_lowering=False, debug=debug,
                   num_devices=NCORES)
    # fp16 everywhere a matmul operand lives: same 1 cyc/row PE rate as
    # f32r but half the DMA/SBUF traffic and far lower PE power (less
    # DVFS throttling).  PSUM stays f32.
    d_state1 = nc.dram_tensor("state1", (DP, 1024), dt, kind="ExternalInput")
    d_gp = nc.dram_tensor("gp", (RL, DP, 3 * GW), dt, kind="ExternalInput")
    d_gf = nc.dram_tensor("gf", (RL, 32, 9 * DF), dt, kind="ExternalInput")
    d_gff = nc.dram_tensor("gffull", (6, DP, 3 * GW), dt, kind="ExternalInput")
    d_m1 = nc.dram_tensor("meas1", (DP, RL * 24), dt, kind="ExternalInput")
    d_m2 = nc.dram_tensor("meas2", (DP, RL * 8), dt, kind="ExternalInput")
    d_ctn = nc.dram_tensor("ctabN", (DP, 1024), F32, kind="ExternalInput")
    d_id = nc.dram_tensor("ident", (8, 8), dt, kind="ExternalInput")
    d_out = nc.dram_tensor("out", (24, 32), F32, kind="ExternalOutput")

    MUL = mybir.AluOpType.mult
    SQ = mybir.ActivationFunctionType.Square

    with tile.TileContext(nc) as tc:
        from contextlib import ExitStack
        with ExitStack() as ex:
            cp = ex.enter_context(tc.tile_pool(name="const", bufs=1))
            sp = ex.enter_context(tc.tile_pool(name="work", bufs=1))
            pp = ex.enter_context(tc.tile_pool(name="ps", bufs=1, space="PSUM"))

            # constants -> SBUF once
            c_state1 = cp.tile([DP, 1024], dt, name="state1", tag="state1")
            c_ctn = cp.tile([DP, 1024], F32, name="ctn", tag="ctn")
            c_id = cp.tile([8, 8], dt, name="ident", tag="ident")
            c_m1 = cp.tile([DP, RL * 24], dt, name="m1", tag="m1")
            c_m2 = cp.tile([DP, RL * 8], dt, name="m2", tag="m2")
            # consts ride on the ACT queue's DGE ring so the sync+gpsimd
            # rings are free for the per-rep gate streams; need-order.
            nc.scalar.dma_start(c_state1[:, 0:512], d_state1.ap()[:, 0:512])
            nc.scalar.dma_start(c_state1[:, 512:1024], d_state1.ap()[:, 512:1024])
            nc.scalar.dma_start(c_ctn, d_ctn.ap())
            nc.scalar.dma_start(c_m2, d_m2.ap())
            nc.scalar.dma_start(c_m1, d_m1.ap())
            nc.scalar.dma_start(c_id, d_id.ap())

            macc = pp.tile([24, 32], F32, name="macc", tag="macc")
            macc_n = [0]

            def emit_rep(r, ch):
                """Return list of 16 stage closures for trajectory r, chain ch."""
                t = {}
                g = f"{ch}"

                def s_dma():
                    t["gp"] = sp.tile([DP, 3 * GW], dt, name="gp", tag="gp" + g, bufs=2)
                    t["gf"] = sp.tile([DP, 3 * GW], dt, name="gf", tag="gf" + g, bufs=2)
                    for l3 in range(3):
                        cs = slice(l3 * GW, (l3 + 1) * GW)
                        eng = nc.sync if l3 < 2 else nc.gpsimd
                        eng.dma_start(t["gp"][:, cs], d_gp.ap()[r][:, cs])
                    # gf planes are kron(I4, .).  The first use of each
                    # physical buffer streams the full planes (seeding the
                    # off-diagonal zeros); every later rep only scatters the
                    # 32-row content into the 4 diagonal blocks (25% of the
                    # bytes).
                    gft = t["gf"]
                    if r in _FULL_GF_REPS:
                        fi = _FULL_GF_REPS.index(r)
                        for l3 in range(3):
                            cs = slice(l3 * GW, (l3 + 1) * GW)
                            nc.gpsimd.dma_start(gft[:, cs], d_gff.ap()[fi][:, cs])
                    else:
                        srf = d_gf.ap()[r]
                        APc, VP = type(gft), type(gft.ap)
                        for bl in range(4):
                            dst = APc(tensor=gft.tensor,
                                      offset=gft.offset + bl * 32 * 1152 + bl * 32,
                                      ap=VP([[1152, 32], [128, 9], [1, 32]]))
                            srcb = APc(tensor=srf.tensor, offset=srf.offset,
                                       ap=VP([[288, 32], [32, 9], [1, 32]]))
                            nc.gpsimd.dma_start(dst, srcb)

                def new_mm():
                    return pp.tile([DP, 512], F32, name="mm", tag="mm" + g, bufs=2)

                def mk_half(stat_key, side, lidx, half, move):
                    """One half-stage: 4 matmuls (b_hi pair 2*half, 2*half+1)
                    into a single-bank [128,512] psum tile, then this half's
                    PSUM->SBUF move.  Stationary = state cols of the global
                    b_hi block; moving = the 256-col gate pair windows."""
                    def s():
                        mm = new_mm()
                        stat = c_state1 if stat_key is None else t[stat_key]
                        mov = t["gp"] if side == "P" else t["gf"]
                        base = lidx * GW
                        movA = mov[:, base + 128:base + 384]
                        movB = mov[:, base:base + 256]
                        for j, bh in enumerate((2 * half, 2 * half + 1)):
                            o, so = j * 256, bh * 256
                            nc.tensor.matmul(mm[:, o:o + 256],
                                             stat[:, so:so + 128], movA,
                                             start=True, stop=False)
                            nc.tensor.matmul(mm[:, o:o + 256],
                                             stat[:, so + 128:so + 256], movB,
                                             start=False, stop=True)
                        move(mm, half)
                    return s

                def mv_copy(key):
                    # h0 on ACT, h1 on DVE: each a single 512-col move with a
                    # multi-half-stage window before the bank is needed again.
                    def m(mm, half):
                        if half == 0:
                            t[key] = sp.tile([DP, 1024], dt, name="tmp",
                                             tag="tmp" + g, bufs=2)
                            nc.scalar.copy(t[key][:, 0:512], mm)
                        else:
                            nc.vector.tensor_copy(t[key][:, 512:1024], mm)
                    return m

                def mv_cmul(key):
                    # x C on DVE via the C table (ACT cannot tensor_tensor)
                    def m(mm, half):
                        if half == 0:
                            t[key] = sp.tile([DP, 1024], dt, name="st",
                                             tag="st" + g, bufs=2)
                        cs = slice(half * 512, (half + 1) * 512)
                        nc.vector.tensor_tensor(t[key][:, cs], mm, c_ctn[:, cs], MUL)
                    return m

                def mv_square(mm, half):
                    # final layer: the move IS the square (contiguous layout,
                    # cols (bh, ri, b_lo, f) for bh pair of this half)
                    key = "sqA" if half == 0 else "sqB"
                    t[key] = sp.tile([DP, 512], dt, name=key, tag=key + g, bufs=2)
                    nc.scalar.activation(t[key], mm, SQ)

                def s_meas1():
                    # m1p rides the chain's own mm-bank rotation: write into
                    # partitions 0:8 of a full-bank allocation.
                    mp = new_mm()
                    m1p = mp[0:8, :]
                    lhs = c_m2[:, r * 8:(r + 1) * 8]
                    for half in range(2):
                        sq = t["sqA"] if half == 0 else t["sqB"]
                        APc, VP = type(sq), type(sq.ap)
                        for ri in range(2):
                            mov = APc(tensor=sq.tensor,
                                      offset=sq.offset + ri * 128,
                                      ap=VP([[512, DP], [256, 2], [1, 128]]))
                            nc.tensor.matmul(m1p[:, half * 256:(half + 1) * 256],
                                             lhs, mov,
                                             start=(ri == 0), stop=(ri == 1))
                    t["m1s"] = sp.tile([8, 512], dt, name="m1s", tag="m1s" + g,
                                       bufs=2)
                    nc.vector.tensor_copy(t["m1s"], m1p)

                def s_meas_tr():
                    # transpose 4 [8,128] blocks -> trm [128, (bh,q)=32];
                    # trm also lives in the chain's mm-bank rotation.
                    trt = new_mm()
                    trm = trt[:, 0:16].bitcast(dt)
                    for bh in range(4):
                        nc.tensor.matmul(trm[:, bh * 8:bh * 8 + 8],
                                         t["m1s"][0:8, bh * 128:(bh + 1) * 128],
                                         c_id, is_transpose=True,
                                         start=(bh == 0), stop=(bh == 3))
                    t["trs"] = sp.tile([DP, 32], dt, name="trs", tag="trs" + g,
                                       bufs=2)
                    nc.vector.tensor_copy(t["trs"], trm)

                def s_macc():
                    macc_n[0] += 1
                    nc.tensor.matmul(macc, c_m1[:, r * 24:(r + 1) * 24], t["trs"],
                                     start=(macc_n[0] == 1),
                                     stop=(macc_n[0] == RL))

                st = [s_dma]
                plan = [(None, "P", 0, mv_copy("t1")),
                        ("t1", "F", 0, mv_cmul("s1")),
                        ("s1", "P", 1, mv_copy("t2")),
                        ("t2", "F", 1, mv_cmul("s2")),
                        ("s2", "P", 2, mv_copy("t3")),
                        ("t3", "F", 2, mv_square)]
                for stat_key, side, lidx, move in plan:
                    st.append(mk_half(stat_key, side, lidx, 0, move))
                    st.append(mk_half(stat_key, side, lidx, 1, move))
                st += [s_meas1, s_meas_tr, s_macc]
                return st

            # rep 0 runs SOLO at the head (the PE is DMA-bound there anyway);
            # then three chains cover reps 1..24, round-robin one stage at a
            # time so every engine sees a steady interleave.
            chains = [
                [emit_rep(rr, ch) for rr in range(1 + ch, RL, 3)]
                for ch in range(3)
            ]
            solo = emit_rep(0, 0)
            for s in solo:
                s()
            queues = [[s for rep in chain for s in rep] for chain in chains]
            pos = [0, 0, 0]
            while any(pos[i] < len(queues[i]) for i in range(3)):
                for i in range(3):
                    if pos[i] < len(queues[i]):
                        queues[i][pos[i]]()
                        pos[i] += 1

            # final: copy accumulator to SBUF, DMA out
            outs = sp.tile([24, 32], F32, name="outs", tag="outs")
            nc.vector.tensor_copy(outs, macc)
            nc.sync.dma_start(d_out.ap(), outs)

    nc.compile()
    return nc


# ---------------- public entry ----------------
_CACHE = {}


def _get_nc():
    if "nc" not in _CACHE:
        _CACHE["nc"] = build_nc()
    return _CACHE["nc"]


def run(inputs, trace=False):
    shared, percore = host_prep(inputs["data_angles"], inputs["params"],
                                inputs["noise_choices"])
    nc = _get_nc()
    in_maps = []
    for c in range(NCORES):
        m = dict(shared)
        m.update(percore[c])
        in_maps.append(m)
    res = bass_utils.run_bass_kernel_spmd(nc, in_maps, list(range(NCORES)),
                                          trace=trace)
    acc = np.zeros((24, 32), np.float64)
    for c in range(NCORES):
        acc += np.asarray(res.results[c]["out"], np.float64)
    return decode_output(acc), res


def kernel(**inputs):
    out, _ = run(inputs)
    return out


# revision 20
# speedup vs baseline: 1.2587x; 1.0972x over previous
"""Trainium2 Bass kernel for NoisyPQC (12-qubit noisy PQC expectation values).

Math restructure (validated vs reference in numpy):
  state index s = p*32 + f;  p = qubits 0..6 (qubit 0 = MSB of p),
  f = qubits 7..11.  state.reshape(128,32)[p,f] == state[s].
  Per trajectory r:  x = B3 D B2 D B1 D B0 psi0 with
    B0 = U0, Bl = Ul*Perm(m_{l-1})*Diag(sigma_{l-1}),
    D = (D_P (x) D_F) * C,  C[p,f] = (-1)^((p&1)*((f>>4)&1)).
  D_P/D_F fold into columns of B_l (l>=1); C applied elementwise 3x.
  Layer 0 is trajectory-independent -> host precomputes
    state1 = C * (GP0 @ psi0 @ GF0^T)  for all 16 batches.
  Device per (r): 3 layers of  phi = GP_l @ phi @ GF_l^T  (x C for l<3),
  then probs = |phi|^2, contracted with z-sign tables; final-layer noise
  becomes sign flips folded into the measurement matrices.

v4 scheme ("stationary-state" matmuls as v2, plus):
  - gate tables store [-B|A|B] (1152 cols/rep/tensor): the two moving
    pair planes [A|B] and [-B|A] are overlapping 256-col windows, so the
    HBM gate stream drops 25%.
  - every gate stage is TWO half-stages of 512 psum cols (one bank);
    per-chain mm tag with bufs=2 decouples the PE from the PSUM->SBUF
    moves: the PE runs ahead into the other bank while a move drains.
  - THREE interleaved chains (reps 1..24) + rep 0 solo at the head where
    the PE is DMA-bound anyway.  Each chain's meas1/transpose PSUM lives
    inside its own mm-bank rotation -> 7 banks total, no cross-chain
    PSUM contention.
  - moves: h0 copy on ACT / h1 copy on DVE; xC multiplies on DVE (h0+h1
    via the C table); final-layer moves ARE the ACT squares (PSUM->SBUF
    with Square), no separate square pass, contiguous (not ri-major) -
    meas1 instead sums ri via two strided-moving matmuls per half.

Layouts (b = 4*b_hi + b_lo):
  N: [p=128 part, col = b_hi*256 + ri*128 + b_lo*32 + f]   (ri: 0=Re,1=Im)
  T: [b_lo*32+f = 128 part, col = b_hi*256 + ri*128 + p]
P-stage (N->T), per b_hi: mm pairA=[GPr^T|GPi^T] then pairB=[-GPi^T|GPr^T]
accumulated.  F-stage (T->N) same with kron(I4, .) planes.  All mms
stream 256 cols => full-speed f32r (1 cyc/row).

Sharding: 200 trajectories = 8 cores x 25. Each core accumulates its 25
trajectories' (sign-flipped) measurement vectors into one PSUM bank via a
single open accumulation group; host sums the 8 [24,32] outputs and /200.
"""

import sys

for _p in ("/opt/trn_rl_repo",):
    if _p not in sys.path:
        sys.path.insert(0, _p)

import numpy as np

from concourse import bacc, bass_utils, mybir
import concourse.tile as tile

# ---------------- problem constants (hardcoded per contract) ----------------
# reps whose gf tile is that physical buffer's FIRST use (full-plane DMA,
# which also seeds the off-diagonal zeros); all later reps only scatter the
# 4 diagonal kron blocks.
_FULL_GF_REPS = [0, 1, 2, 3, 5, 6]
NQ = 12
NL = 4
REPS = 200
BATCH = 16
NCORES = 8
RL = REPS // NCORES  # 25 reps per core
DP, DF = 128, 32  # dim of p-side (qubits 0..6) and f-side (qubits 7..11)
GW = 384  # gate cols per layer: [-B|A|B]; pairB = cols 0:256, pairA = 128:384

F32 = mybir.dt.float32
F32R = mybir.dt.float32r
FP16 = mybir.dt.float16


# ---------------- host-side math ----------------
def _rx(t):
    c, s = np.cos(0.5 * t), -1j * np.sin(0.5 * t)
    return np.array([[c, s], [s, c]], np.complex64)


def _rz(t):
    return np.array([[np.exp(-0.5j * t), 0], [0, np.exp(0.5j * t)]], np.complex64)


def _kron_list(mats):
    out = mats[0]
    for m in mats[1:]:
        out = np.kron(out, m)
    return out


def _tables():
    p = np.arange(DP)
    f = np.arange(DF)
    dP = np.ones(DP)
    for j in range(6):
        dP *= np.where(((p >> (6 - j)) & 1) & ((p >> (5 - j)) & 1), -1.0, 1.0)
    dF = np.ones(DF)
    for k in range(4):
        dF *= np.where(((f >> (4 - k)) & 1) & ((f >> (3 - k)) & 1), -1.0, 1.0)
    C = np.where(((p[:, None] & 1) & ((f[None, :] >> 4) & 1)) == 1, -1.0, 1.0)
    zP = 1.0 - 2.0 * ((p[None, :] >> (6 - np.arange(7)[:, None])) & 1)  # [7,128]
    zF = 1.0 - 2.0 * ((f[None, :] >> (4 - np.arange(5)[:, None])) & 1)  # [5,32]
    return dP, dF, C, zP, zF


def host_prep(data_angles, params, noise):
    """Build all device arrays. Returns (shared dict, per-core list of dicts)."""
    da = np.asarray(data_angles, np.float64)
    pa = np.asarray(params, np.float64)
    nz = np.asarray(noise)
    dPt, dFt, C, zP, zF = _tables()

    # base per-qubit gates u[l][q] = Rx(params[l,q,1]) @ Rz(params[l,q,0])
    u = [[_rx(pa[l, q, 1]) @ _rz(pa[l, q, 0]) for q in range(NQ)] for l in range(NL)]

    # --- state after layer 0 (incl. C), identical for every trajectory ---
    va = np.stack([np.cos(0.5 * da), -1j * np.sin(0.5 * da)], -1)  # [B,12,2]
    GP0 = _kron_list([u[0][q] for q in range(7)])        # [128,128]
    GF0 = _kron_list([u[0][q] for q in range(7, NQ)])    # [32,32]
    s_re = np.empty((DP, BATCH * DF), np.float32)
    s_im = np.empty((DP, BATCH * DF), np.float32)
    for b in range(BATCH):
        vPr = _kron_list([va[b, q].astype(np.complex128) for q in range(7)])
        vFr = _kron_list([va[b, q].astype(np.complex128) for q in range(7, NQ)])
        phi = (GP0 @ np.outer(vPr, vFr) @ GF0.T) * C
        s_re[:, b * DF:(b + 1) * DF] = phi.real
        s_im[:, b * DF:(b + 1) * DF] = phi.imag
    # N layout: col = b_hi*256 + ri*128 + b_lo*32 + f
    state1 = np.empty((DP, 1024), np.float32)
    for bh in range(4):
        state1[:, bh * 256:bh * 256 + 128] = s_re[:, bh * 128:bh * 128 + 128]
        state1[:, bh * 256 + 128:bh * 256 + 256] = s_im[:, bh * 128:bh * 128 + 128]

    # --- per-core moving gate tables + measurement tables ---
    # per layer block (GW=384 cols): [-B | A | B] with A = Re-plane, B =
    # Im-plane; pairA=[A|B] = cols 128:384, pairB=[-B|A] = cols 0:256.
    eye4 = np.eye(4)
    percore = []
    for c in range(NCORES):
        gp = np.empty((RL, DP, 3 * GW), np.float32)
        gf = np.empty((RL, 32, 9 * DF), np.float32)
        gff = np.zeros((6, DP, 3 * GW), np.float32)  # full planes, first-buffer reps
        m1 = np.empty((DP, RL * 24), np.float32)
        m2 = np.empty((DP, RL * 8), np.float32)
        for rl in range(RL):
            r = c * RL + rl
            for lidx, l in enumerate((1, 2, 3)):
                facs = []
                for q in range(NQ):
                    g = u[l][q]
                    ch = int(nz[r, l - 1, q])
                    if ch in (1, 2):
                        g = g[:, ::-1]
                    if ch in (2, 3):
                        g = g * np.array([1.0, -1.0])[None, :]
                    facs.append(g)
                GP = _kron_list(facs[:7]) * dPt[None, :]
                GF = _kron_list(facs[7:]) * dFt[None, :]
                o = lidx * GW
                gp[rl, :, o:o + 128] = -GP.imag.T
                gp[rl, :, o + 128:o + 256] = GP.real.T
                gp[rl, :, o + 256:o + 384] = GP.imag.T
                of = lidx * 3 * DF
                gf[rl, :, of:of + DF] = -GF.imag.T
                gf[rl, :, of + DF:of + 2 * DF] = GF.real.T
                gf[rl, :, of + 2 * DF:of + 3 * DF] = GF.imag.T
                if rl in _FULL_GF_REPS:
                    fi = _FULL_GF_REPS.index(rl)
                    A = np.kron(eye4, GF.real.T)
                    Bm = np.kron(eye4, GF.imag.T)
                    gff[fi, :, o:o + 128] = -Bm
                    gff[fi, :, o + 128:o + 256] = A
                    gff[fi, :, o + 256:o + 384] = Bm
            m3 = nz[r, NL - 1]  # [12]
            flip = np.where((m3 == 1) | (m3 == 2), -1.0, 1.0)
            M1 = np.ones((DF, 6))
            M1[:, :5] = (zF * flip[7:, None]).T  # [32,5]
            m1[:, rl * 24:(rl + 1) * 24] = np.kron(eye4, M1)
            M2 = np.ones((DP, 8), np.float64)
            M2[:, :7] = (zP * flip[:7, None]).T
            m2[:, rl * 8:(rl + 1) * 8] = M2
        percore.append({"gp": gp, "gf": gf, "gffull": gff, "meas1": m1, "meas2": m2})

    shared = {
        "state1": np.ascontiguousarray(state1),
        "ctabN": np.ascontiguousarray(np.tile(C, (1, 32)).astype(np.float32)),
        "ident": np.eye(8, dtype=np.float32),
    }
    for d in percore:
        for k in list(d):
            d[k] = np.ascontiguousarray(d[k], np.float16)
    for k in ("state1", "ident"):
        shared[k] = np.ascontiguousarray(shared[k], np.float16)
    return shared, percore


def decode_output(acc):
    """acc: [24,32] summed over cores. Returns [16,12] float32."""
    out = np.empty((BATCH, NQ), np.float64)
    for bh in range(4):
        for bl in range(4):
            b = 4 * bh + bl
            for q in range(7):
                out[b, q] = acc[bl * 6 + 5, bh * 8 + q]
            for j in range(5):
                out[b, 7 + j] = acc[bl * 6 + j, bh * 8 + 7]
    return (out / REPS).astype(np.float32)


# ---------------- device kernel ----------------
def build_nc(dt=FP16, debug=False):
    """Build + compile the per-core Bass program (same for all cores)."""
    nc = bacc.Bacc("TRN2", target_bir_lowering=False, debug=debug,
                   num_devices=NCORES)
    # fp16 everywhere a matmul operand lives: same 1 cyc/row PE rate as
    # f32r but half the DMA/SBUF traffic and far lower PE power (less
    # DVFS throttling).  PSUM stays f32.
    d_state1 = nc.dram_tensor("state1", (DP, 1024), dt, kind="ExternalInput")
    d_gp = nc.dram_tensor("gp", (RL, DP, 3 * GW), dt, kind="ExternalInput")
    d_gf = nc.dram_tensor("gf", (RL, 32, 9 * DF), dt, kind="ExternalInput")
    d_gff = nc.dram_tensor("gffull", (6, DP, 3 * GW), dt, kind="ExternalInput")
    d_m1 = nc.dram_tensor("meas1", (DP, RL * 24), dt, kind="ExternalInput")
    d_m2 = nc.dram_tensor("meas2", (DP, RL * 8), dt, kind="ExternalInput")
    d_ctn = nc.dram_tensor("ctabN", (DP, 1024), F32, kind="ExternalInput")
    d_id = nc.dram_tensor("ident", (8, 8), dt, kind="ExternalInput")
    d_out = nc.dram_tensor("out", (24, 32), F32, kind="ExternalOutput")

    MUL = mybir.AluOpType.mult
    SQ = mybir.ActivationFunctionType.Square

    with tile.TileContext(nc) as tc:
        from contextlib import ExitStack
        with ExitStack() as ex:
            cp = ex.enter_context(tc.tile_pool(name="const", bufs=1))
            sp = ex.enter_context(tc.tile_pool(name="work", bufs=1))
            pp = ex.enter_context(tc.tile_pool(name="ps", bufs=1, space="PSUM"))

            # constants -> SBUF once
            c_state1 = cp.tile([DP, 1024], dt, name="state1", tag="state1")
            c_ctn = cp.tile([DP, 1024], F32, name="ctn", tag="ctn")
            c_id = cp.tile([8, 8], dt, name="ident", tag="ident")
            c_m1 = cp.tile([DP, RL * 24], dt, name="m1", tag="m1")
            c_m2 = cp.tile([DP, RL * 8], dt, name="m2", tag="m2")
            # consts ride on the ACT queue's DGE ring so the sync+gpsimd
            # rings are free for the per-rep gate streams; need-order.
            nc.scalar.dma_start(c_state1[:, 0:512], d_state1.ap()[:, 0:512])
            nc.scalar.dma_start(c_state1[:, 512:1024], d_state1.ap()[:, 512:1024])
            nc.scalar.dma_start(c_ctn, d_ctn.ap())
            nc.scalar.dma_start(c_m2, d_m2.ap())
            nc.scalar.dma_start(c_m1, d_m1.ap())
            nc.scalar.dma_start(c_id, d_id.ap())

            macc = pp.tile([24, 32], F32, name="macc", tag="macc")
            macc_n = [0]

            def emit_rep(r, ch):
                """Return list of 16 stage closures for trajectory r, chain ch."""
                t = {}
                g = f"{ch}"

                def s_dma():
                    t["gp"] = sp.tile([DP, 3 * GW], dt, name="gp", tag="gp" + g, bufs=2)
                    t["gf"] = sp.tile([DP, 3 * GW], dt, name="gf", tag="gf" + g, bufs=2)
                    for l3 in range(3):
                        cs = slice(l3 * GW, (l3 + 1) * GW)
                        eng = nc.sync if l3 < 2 else nc.gpsimd
                        eng.dma_start(t["gp"][:, cs], d_gp.ap()[r][:, cs])
                    # gf planes are kron(I4, .).  The first use of each
                    # physical buffer streams the full planes (seeding the
                    # off-diagonal zeros); every later rep only scatters the
                    # 32-row content into the 4 diagonal blocks (25% of the
                    # bytes).
                    gft = t["gf"]
                    if r in _FULL_GF_REPS:
                        fi = _FULL_GF_REPS.index(r)
                        for l3 in range(3):
                            cs = slice(l3 * GW, (l3 + 1) * GW)
                            nc.gpsimd.dma_start(gft[:, cs], d_gff.ap()[fi][:, cs])
                    else:
                        srf = d_gf.ap()[r]
                        APc, VP = type(gft), type(gft.ap)
                        for bl in range(4):
                            dst = APc(tensor=gft.tensor,
                                      offset=gft.offset + bl * 32 * 1152 + bl * 32,
                                      ap=VP([[1152, 32], [128, 9], [1, 32]]))
                            srcb = APc(tensor=srf.tensor, offset=srf.offset,
                                       ap=VP([[288, 32], [32, 9], [1, 32]]))
                            nc.gpsimd.dma_start(dst, srcb)

                def new_mm():
                    return pp.tile([DP, 512], F32, name="mm", tag="mm" + g, bufs=2)

                def mk_half(stat_key, side, lidx, half, move):
                    """One half-stage: 4 matmuls (b_hi pair 2*half, 2*half+1)
                    into a single-bank [128,512] psum tile, then this half's
                    PSUM->SBUF move.  Stationary = state cols of the global
                    b_hi block; moving = the 256-col gate pair windows."""
                    def s():
                        mm = new_mm()
                        stat = c_state1 if stat_key is None else t[stat_key]
                        mov = t["gp"] if side == "P" else t["gf"]
                        base = lidx * GW
                        movA = mov[:, base + 128:base + 384]
                        movB = mov[:, base:base + 256]
                        for j, bh in enumerate((2 * half, 2 * half + 1)):
                            o, so = j * 256, bh * 256
                            nc.tensor.matmul(mm[:, o:o + 256],
                                             stat[:, so:so + 128], movA,
                                             start=True, stop=False)
                            nc.tensor.matmul(mm[:, o:o + 256],
                                             stat[:, so + 128:so + 256], movB,
                                             start=False, stop=True)
                        move(mm, half)
                    return s

                def mv_copy(key):
                    # h0 on ACT, h1 on DVE: each a single 512-col move with a
                    # multi-half-stage window before the bank is needed again.
                    def m(mm, half):
                        if half == 0:
                            t[key] = sp.tile([DP, 1024], dt, name="tmp",
                                             tag="tmp" + g, bufs=2)
                            nc.scalar.copy(t[key][:, 0:512], mm)
                        else:
                            nc.vector.tensor_copy(t[key][:, 512:1024], mm)
                    return m

                def mv_cmul(key):
                    # x C on DVE via the C table (ACT cannot tensor_tensor)
                    def m(mm, half):
                        if half == 0:
                            t[key] = sp.tile([DP, 1024], dt, name="st",
                                             tag="st" + g, bufs=2)
                        cs = slice(half * 512, (half + 1) * 512)
                        nc.vector.tensor_tensor(t[key][:, cs], mm, c_ctn[:, cs], MUL)
                    return m

                def mv_square(mm, half):
                    # final layer: the move IS the square (contiguous layout,
                    # cols (bh, ri, b_lo, f) for bh pair of this half)
                    key = "sqA" if half == 0 else "sqB"
                    t[key] = sp.tile([DP, 512], dt, name=key, tag=key + g, bufs=2)
                    nc.scalar.activation(t[key], mm, SQ)

                def s_meas1():
                    # p-contraction with the OUTPUT TRANSPOSED: sq is the
                    # stationary operand, the per-rep m2 column block the
                    # moving one, so out partitions = sq columns = (bl,f) and
                    # out cols = q.  fp16 matmuls run 1 cyc/row at any width,
                    # so 8 narrow matmuls replace meas1 + 4 PE transposes
                    # (and the transpose-mode pipeline drains).
                    mp = new_mm()
                    trm = mp[:, 0:32]
                    mov = c_m2[:, r * 8:(r + 1) * 8]
                    for bh in range(4):
                        sq = t["sqA"] if bh < 2 else t["sqB"]
                        for ri in range(2):
                            blk = (bh % 2) * 256 + ri * 128
                            nc.tensor.matmul(trm[:, bh * 8:bh * 8 + 8],
                                             sq[:, blk:blk + 128], mov,
                                             start=(ri == 0), stop=(ri == 1))
                    t["trs"] = sp.tile([DP, 32], dt, name="trs", tag="trs" + g,
                                       bufs=2)
                    nc.vector.tensor_copy(t["trs"], trm)

                def s_macc():
                    macc_n[0] += 1
                    nc.tensor.matmul(macc, c_m1[:, r * 24:(r + 1) * 24], t["trs"],
                                     start=(macc_n[0] == 1),
                                     stop=(macc_n[0] == RL))

                st = [s_dma]
                plan = [(None, "P", 0, mv_copy("t1")),
                        ("t1", "F", 0, mv_cmul("s1")),
                        ("s1", "P", 1, mv_copy("t2")),
                        ("t2", "F", 1, mv_cmul("s2")),
                        ("s2", "P", 2, mv_copy("t3")),
                        ("t3", "F", 2, mv_square)]
                for stat_key, side, lidx, move in plan:
                    st.append(mk_half(stat_key, side, lidx, 0, move))
                    st.append(mk_half(stat_key, side, lidx, 1, move))
                st += [s_meas1, s_macc]
                return st

            # rep 0 runs SOLO at the head (the PE is DMA-bound there anyway);
            # then three chains cover reps 1..24, round-robin one stage at a
            # time so every engine sees a steady interleave.
            chains = [
                [emit_rep(rr, ch) for rr in range(1 + ch, RL, 3)]
                for ch in range(3)
            ]
            solo = emit_rep(0, 0)
            for s in solo:
                s()
            queues = [[s for rep in chain for s in rep] for chain in chains]
            pos = [0, 0, 0]
            while any(pos[i] < len(queues[i]) for i in range(3)):
                for i in range(3):
                    if pos[i] < len(queues[i]):
                        queues[i][pos[i]]()
                        pos[i] += 1

            # final: copy accumulator to SBUF, DMA out
            outs = sp.tile([24, 32], F32, name="outs", tag="outs")
            nc.vector.tensor_copy(outs, macc)
            nc.sync.dma_start(d_out.ap(), outs)

    nc.compile()
    return nc


# ---------------- public entry ----------------
_CACHE = {}


def _get_nc():
    if "nc" not in _CACHE:
        _CACHE["nc"] = build_nc()
    return _CACHE["nc"]


def run(inputs, trace=False):
    shared, percore = host_prep(inputs["data_angles"], inputs["params"],
                                inputs["noise_choices"])
    nc = _get_nc()
    in_maps = []
    for c in range(NCORES):
        m = dict(shared)
        m.update(percore[c])
        in_maps.append(m)
    res = bass_utils.run_bass_kernel_spmd(nc, in_maps, list(range(NCORES)),
                                          trace=trace)
    acc = np.zeros((24, 32), np.float64)
    for c in range(NCORES):
        acc += np.asarray(res.results[c]["out"], np.float64)
    return decode_output(acc), res


def kernel(**inputs):
    out, _ = run(inputs)
    return out


# revision 21
# speedup vs baseline: 1.2698x; 1.0088x over previous
"""Trainium2 Bass kernel for NoisyPQC (12-qubit noisy PQC expectation values).

Math restructure (validated vs reference in numpy):
  state index s = p*32 + f;  p = qubits 0..6 (qubit 0 = MSB of p),
  f = qubits 7..11.  state.reshape(128,32)[p,f] == state[s].
  Per trajectory r:  x = B3 D B2 D B1 D B0 psi0 with
    B0 = U0, Bl = Ul*Perm(m_{l-1})*Diag(sigma_{l-1}),
    D = (D_P (x) D_F) * C,  C[p,f] = (-1)^((p&1)*((f>>4)&1)).
  D_P/D_F fold into columns of B_l (l>=1); C applied elementwise 3x.
  Layer 0 is trajectory-independent -> host precomputes
    state1 = C * (GP0 @ psi0 @ GF0^T)  for all 16 batches.
  Device per (r): 3 layers of  phi = GP_l @ phi @ GF_l^T  (x C for l<3),
  then probs = |phi|^2, contracted with z-sign tables; final-layer noise
  becomes sign flips folded into the measurement matrices.

v4 scheme ("stationary-state" matmuls as v2, plus):
  - gate tables store [-B|A|B] (1152 cols/rep/tensor): the two moving
    pair planes [A|B] and [-B|A] are overlapping 256-col windows, so the
    HBM gate stream drops 25%.
  - every gate stage is TWO half-stages of 512 psum cols (one bank);
    per-chain mm tag with bufs=2 decouples the PE from the PSUM->SBUF
    moves: the PE runs ahead into the other bank while a move drains.
  - THREE interleaved chains (reps 1..24) + rep 0 solo at the head where
    the PE is DMA-bound anyway.  Each chain's meas1/transpose PSUM lives
    inside its own mm-bank rotation -> 7 banks total, no cross-chain
    PSUM contention.
  - moves: h0 copy on ACT / h1 copy on DVE; xC multiplies on DVE (h0+h1
    via the C table); final-layer moves ARE the ACT squares (PSUM->SBUF
    with Square), no separate square pass, contiguous (not ri-major) -
    meas1 instead sums ri via two strided-moving matmuls per half.

Layouts (b = 4*b_hi + b_lo):
  N: [p=128 part, col = b_hi*256 + ri*128 + b_lo*32 + f]   (ri: 0=Re,1=Im)
  T: [b_lo*32+f = 128 part, col = b_hi*256 + ri*128 + p]
P-stage (N->T), per b_hi: mm pairA=[GPr^T|GPi^T] then pairB=[-GPi^T|GPr^T]
accumulated.  F-stage (T->N) same with kron(I4, .) planes.  All mms
stream 256 cols => full-speed f32r (1 cyc/row).

Sharding: 200 trajectories = 8 cores x 25. Each core accumulates its 25
trajectories' (sign-flipped) measurement vectors into one PSUM bank via a
single open accumulation group; host sums the 8 [24,32] outputs and /200.
"""

import sys

for _p in ("/opt/trn_rl_repo",):
    if _p not in sys.path:
        sys.path.insert(0, _p)

import numpy as np

from concourse import bacc, bass_utils, mybir
import concourse.tile as tile

# ---------------- problem constants (hardcoded per contract) ----------------
NQ = 12
NL = 4
REPS = 200
BATCH = 16
NCORES = 8
RL = REPS // NCORES  # 25 reps per core
DP, DF = 128, 32  # dim of p-side (qubits 0..6) and f-side (qubits 7..11)
GW = 384  # gate cols per layer: [-B|A|B]; pairB = cols 0:256, pairA = 128:384

F32 = mybir.dt.float32
F32R = mybir.dt.float32r
FP16 = mybir.dt.float16


# ---------------- host-side math ----------------
def _rx(t):
    c, s = np.cos(0.5 * t), -1j * np.sin(0.5 * t)
    return np.array([[c, s], [s, c]], np.complex64)


def _rz(t):
    return np.array([[np.exp(-0.5j * t), 0], [0, np.exp(0.5j * t)]], np.complex64)


def _kron_list(mats):
    out = mats[0]
    for m in mats[1:]:
        out = np.kron(out, m)
    return out


def _tables():
    p = np.arange(DP)
    f = np.arange(DF)
    dP = np.ones(DP)
    for j in range(6):
        dP *= np.where(((p >> (6 - j)) & 1) & ((p >> (5 - j)) & 1), -1.0, 1.0)
    dF = np.ones(DF)
    for k in range(4):
        dF *= np.where(((f >> (4 - k)) & 1) & ((f >> (3 - k)) & 1), -1.0, 1.0)
    C = np.where(((p[:, None] & 1) & ((f[None, :] >> 4) & 1)) == 1, -1.0, 1.0)
    zP = 1.0 - 2.0 * ((p[None, :] >> (6 - np.arange(7)[:, None])) & 1)  # [7,128]
    zF = 1.0 - 2.0 * ((f[None, :] >> (4 - np.arange(5)[:, None])) & 1)  # [5,32]
    return dP, dF, C, zP, zF


def host_prep(data_angles, params, noise):
    """Build all device arrays. Returns (shared dict, per-core list of dicts)."""
    da = np.asarray(data_angles, np.float64)
    pa = np.asarray(params, np.float64)
    nz = np.asarray(noise)
    dPt, dFt, C, zP, zF = _tables()

    # base per-qubit gates u[l][q] = Rx(params[l,q,1]) @ Rz(params[l,q,0])
    u = [[_rx(pa[l, q, 1]) @ _rz(pa[l, q, 0]) for q in range(NQ)] for l in range(NL)]

    # --- state after layer 0 (incl. C), identical for every trajectory ---
    va = np.stack([np.cos(0.5 * da), -1j * np.sin(0.5 * da)], -1)  # [B,12,2]
    GP0 = _kron_list([u[0][q] for q in range(7)])        # [128,128]
    GF0 = _kron_list([u[0][q] for q in range(7, NQ)])    # [32,32]
    s_re = np.empty((DP, BATCH * DF), np.float32)
    s_im = np.empty((DP, BATCH * DF), np.float32)
    for b in range(BATCH):
        vPr = _kron_list([va[b, q].astype(np.complex128) for q in range(7)])
        vFr = _kron_list([va[b, q].astype(np.complex128) for q in range(7, NQ)])
        phi = (GP0 @ np.outer(vPr, vFr) @ GF0.T) * C
        s_re[:, b * DF:(b + 1) * DF] = phi.real
        s_im[:, b * DF:(b + 1) * DF] = phi.imag
    # N layout: col = b_hi*256 + ri*128 + b_lo*32 + f
    state1 = np.empty((DP, 1024), np.float32)
    for bh in range(4):
        state1[:, bh * 256:bh * 256 + 128] = s_re[:, bh * 128:bh * 128 + 128]
        state1[:, bh * 256 + 128:bh * 256 + 256] = s_im[:, bh * 128:bh * 128 + 128]

    # --- per-core moving gate tables + measurement tables ---
    # per layer block (GW=384 cols): [-B | A | B] with A = Re-plane, B =
    # Im-plane; pairA=[A|B] = cols 128:384, pairB=[-B|A] = cols 0:256.
    eye4 = np.eye(4)
    percore = []
    for c in range(NCORES):
        gp = np.empty((RL, DP, 3 * GW), np.float32)
        gf = np.empty((RL, 32, 9 * DF), np.float32)
        m1 = np.empty((DP, RL * 24), np.float32)
        m2 = np.empty((DP, RL * 8), np.float32)
        for rl in range(RL):
            r = c * RL + rl
            for lidx, l in enumerate((1, 2, 3)):
                facs = []
                for q in range(NQ):
                    g = u[l][q]
                    ch = int(nz[r, l - 1, q])
                    if ch in (1, 2):
                        g = g[:, ::-1]
                    if ch in (2, 3):
                        g = g * np.array([1.0, -1.0])[None, :]
                    facs.append(g)
                GP = _kron_list(facs[:7]) * dPt[None, :]
                GF = _kron_list(facs[7:]) * dFt[None, :]
                o = lidx * GW
                gp[rl, :, o:o + 128] = -GP.imag.T
                gp[rl, :, o + 128:o + 256] = GP.real.T
                gp[rl, :, o + 256:o + 384] = GP.imag.T
                of = lidx * 3 * DF
                gf[rl, :, of:of + DF] = -GF.imag.T
                gf[rl, :, of + DF:of + 2 * DF] = GF.real.T
                gf[rl, :, of + 2 * DF:of + 3 * DF] = GF.imag.T
            m3 = nz[r, NL - 1]  # [12]
            flip = np.where((m3 == 1) | (m3 == 2), -1.0, 1.0)
            M1 = np.ones((DF, 6))
            M1[:, :5] = (zF * flip[7:, None]).T  # [32,5]
            m1[:, rl * 24:(rl + 1) * 24] = np.kron(eye4, M1)
            M2 = np.ones((DP, 8), np.float64)
            M2[:, :7] = (zP * flip[:7, None]).T
            m2[:, rl * 8:(rl + 1) * 8] = M2
        percore.append({"gp": gp, "gf": gf, "meas1": m1, "meas2": m2})

    shared = {
        "state1": np.ascontiguousarray(state1),
        "ctabN": np.ascontiguousarray(np.tile(C, (1, 32)).astype(np.float32)),
        "ident": np.eye(8, dtype=np.float32),
    }
    for d in percore:
        for k in list(d):
            d[k] = np.ascontiguousarray(d[k], np.float16)
    for k in ("state1", "ident"):
        shared[k] = np.ascontiguousarray(shared[k], np.float16)
    return shared, percore


def decode_output(acc):
    """acc: [24,32] summed over cores. Returns [16,12] float32."""
    out = np.empty((BATCH, NQ), np.float64)
    for bh in range(4):
        for bl in range(4):
            b = 4 * bh + bl
            for q in range(7):
                out[b, q] = acc[bl * 6 + 5, bh * 8 + q]
            for j in range(5):
                out[b, 7 + j] = acc[bl * 6 + j, bh * 8 + 7]
    return (out / REPS).astype(np.float32)


# ---------------- device kernel ----------------
def build_nc(dt=FP16, debug=False):
    """Build + compile the per-core Bass program (same for all cores)."""
    nc = bacc.Bacc("TRN2", target_bir_lowering=False, debug=debug,
                   num_devices=NCORES)
    # fp16 everywhere a matmul operand lives: same 1 cyc/row PE rate as
    # f32r but half the DMA/SBUF traffic and far lower PE power (less
    # DVFS throttling).  PSUM stays f32.
    d_state1 = nc.dram_tensor("state1", (DP, 1024), dt, kind="ExternalInput")
    d_gp = nc.dram_tensor("gp", (RL, DP, 3 * GW), dt, kind="ExternalInput")
    d_gf = nc.dram_tensor("gf", (RL, 32, 9 * DF), dt, kind="ExternalInput")
    d_m1 = nc.dram_tensor("meas1", (DP, RL * 24), dt, kind="ExternalInput")
    d_m2 = nc.dram_tensor("meas2", (DP, RL * 8), dt, kind="ExternalInput")
    d_ctn = nc.dram_tensor("ctabN", (DP, 1024), F32, kind="ExternalInput")
    d_id = nc.dram_tensor("ident", (8, 8), dt, kind="ExternalInput")
    d_out = nc.dram_tensor("out", (24, 32), F32, kind="ExternalOutput")

    MUL = mybir.AluOpType.mult
    SQ = mybir.ActivationFunctionType.Square

    with tile.TileContext(nc) as tc:
        from contextlib import ExitStack
        with ExitStack() as ex:
            cp = ex.enter_context(tc.tile_pool(name="const", bufs=1))
            sp = ex.enter_context(tc.tile_pool(name="work", bufs=1))
            pp = ex.enter_context(tc.tile_pool(name="ps", bufs=1, space="PSUM"))

            # constants -> SBUF once
            c_state1 = cp.tile([DP, 1024], dt, name="state1", tag="state1")
            c_ctn = cp.tile([DP, 1024], F32, name="ctn", tag="ctn")
            c_id = cp.tile([8, 8], dt, name="ident", tag="ident")
            c_m1 = cp.tile([DP, RL * 24], dt, name="m1", tag="m1")
            c_m2 = cp.tile([DP, RL * 8], dt, name="m2", tag="m2")
            # consts ride on the ACT queue's DGE ring so the sync+gpsimd
            # rings are free for the per-rep gate streams; need-order.
            nc.scalar.dma_start(c_state1[:, 0:512], d_state1.ap()[:, 0:512])
            nc.scalar.dma_start(c_state1[:, 512:1024], d_state1.ap()[:, 512:1024])
            nc.scalar.dma_start(c_ctn, d_ctn.ap())
            nc.scalar.dma_start(c_m2, d_m2.ap())
            nc.scalar.dma_start(c_m1, d_m1.ap())
            nc.scalar.dma_start(c_id, d_id.ap())

            macc = pp.tile([24, 32], F32, name="macc", tag="macc")
            macc_n = [0]

            # zero the gf double-buffers once: per-rep DMAs only scatter the
            # 4 diagonal kron blocks.  Two dummy allocations per tag walk
            # each rotation exactly one full cycle.
            for zch in range(3):
                for _ in range(2):
                    z = sp.tile([DP, 3 * GW], dt, name="gfz", tag="gf" + str(zch),
                                bufs=2)
                    nc.vector.memset(z.bitcast(mybir.dt.uint32), 0)

            def emit_rep(r, ch):
                """Return list of 16 stage closures for trajectory r, chain ch."""
                t = {}
                g = f"{ch}"

                def s_dma():
                    t["gp"] = sp.tile([DP, 3 * GW], dt, name="gp", tag="gp" + g, bufs=2)
                    t["gf"] = sp.tile([DP, 3 * GW], dt, name="gf", tag="gf" + g, bufs=2)
                    for l3 in range(3):
                        cs = slice(l3 * GW, (l3 + 1) * GW)
                        eng = nc.sync if l3 < 2 else nc.gpsimd
                        eng.dma_start(t["gp"][:, cs], d_gp.ap()[r][:, cs])
                    # gf planes are kron(I4, .): scatter only the 32-row
                    # content into the 4 diagonal blocks; the off-diagonal
                    # zeros were memset once at startup.
                    gft = t["gf"]
                    srf = d_gf.ap()[r]
                    APc, VP = type(gft), type(gft.ap)
                    for bl in range(4):
                        dst = APc(tensor=gft.tensor,
                                  offset=gft.offset + bl * 32 * 1152 + bl * 32,
                                  ap=VP([[1152, 32], [128, 9], [1, 32]]))
                        srcb = APc(tensor=srf.tensor, offset=srf.offset,
                                   ap=VP([[288, 32], [32, 9], [1, 32]]))
                        nc.gpsimd.dma_start(dst, srcb)

                def new_mm():
                    return pp.tile([DP, 512], F32, name="mm", tag="mm" + g, bufs=2)

                def mk_half(stat_key, side, lidx, half, move):
                    """One half-stage: 4 matmuls (b_hi pair 2*half, 2*half+1)
                    into a single-bank [128,512] psum tile, then this half's
                    PSUM->SBUF move.  Stationary = state cols of the global
                    b_hi block; moving = the 256-col gate pair windows."""
                    def s():
                        mm = new_mm()
                        stat = c_state1 if stat_key is None else t[stat_key]
                        mov = t["gp"] if side == "P" else t["gf"]
                        base = lidx * GW
                        movA = mov[:, base + 128:base + 384]
                        movB = mov[:, base:base + 256]
                        for j, bh in enumerate((2 * half, 2 * half + 1)):
                            o, so = j * 256, bh * 256
                            nc.tensor.matmul(mm[:, o:o + 256],
                                             stat[:, so:so + 128], movA,
                                             start=True, stop=False)
                            nc.tensor.matmul(mm[:, o:o + 256],
                                             stat[:, so + 128:so + 256], movB,
                                             start=False, stop=True)
                        move(mm, half)
                    return s

                def mv_copy(key):
                    # h0 on ACT, h1 on DVE: each a single 512-col move with a
                    # multi-half-stage window before the bank is needed again.
                    def m(mm, half):
                        if half == 0:
                            t[key] = sp.tile([DP, 1024], dt, name="tmp",
                                             tag="tmp" + g, bufs=2)
                            nc.scalar.copy(t[key][:, 0:512], mm)
                        else:
                            nc.vector.tensor_copy(t[key][:, 512:1024], mm)
                    return m

                def mv_cmul(key):
                    # x C on DVE via the C table (ACT cannot tensor_tensor)
                    def m(mm, half):
                        if half == 0:
                            t[key] = sp.tile([DP, 1024], dt, name="st",
                                             tag="st" + g, bufs=2)
                        cs = slice(half * 512, (half + 1) * 512)
                        nc.vector.tensor_tensor(t[key][:, cs], mm, c_ctn[:, cs], MUL)
                    return m

                def mv_square(mm, half):
                    # final layer: the move IS the square (contiguous layout,
                    # cols (bh, ri, b_lo, f) for bh pair of this half)
                    key = "sqA" if half == 0 else "sqB"
                    t[key] = sp.tile([DP, 512], dt, name=key, tag=key + g, bufs=2)
                    nc.scalar.activation(t[key], mm, SQ)

                def s_meas1():
                    # p-contraction with the OUTPUT TRANSPOSED: sq is the
                    # stationary operand, the per-rep m2 column block the
                    # moving one, so out partitions = sq columns = (bl,f) and
                    # out cols = q.  fp16 matmuls run 1 cyc/row at any width,
                    # so 8 narrow matmuls replace meas1 + 4 PE transposes
                    # (and the transpose-mode pipeline drains).
                    mp = new_mm()
                    trm = mp[:, 0:32]
                    mov = c_m2[:, r * 8:(r + 1) * 8]
                    for bh in range(4):
                        sq = t["sqA"] if bh < 2 else t["sqB"]
                        for ri in range(2):
                            blk = (bh % 2) * 256 + ri * 128
                            nc.tensor.matmul(trm[:, bh * 8:bh * 8 + 8],
                                             sq[:, blk:blk + 128], mov,
                                             start=(ri == 0), stop=(ri == 1))
                    t["trs"] = sp.tile([DP, 32], dt, name="trs", tag="trs" + g,
                                       bufs=2)
                    nc.vector.tensor_copy(t["trs"], trm)

                def s_macc():
                    macc_n[0] += 1
                    nc.tensor.matmul(macc, c_m1[:, r * 24:(r + 1) * 24], t["trs"],
                                     start=(macc_n[0] == 1),
                                     stop=(macc_n[0] == RL))

                st = [s_dma]
                plan = [(None, "P", 0, mv_copy("t1")),
                        ("t1", "F", 0, mv_cmul("s1")),
                        ("s1", "P", 1, mv_copy("t2")),
                        ("t2", "F", 1, mv_cmul("s2")),
                        ("s2", "P", 2, mv_copy("t3")),
                        ("t3", "F", 2, mv_square)]
                for stat_key, side, lidx, move in plan:
                    st.append(mk_half(stat_key, side, lidx, 0, move))
                    st.append(mk_half(stat_key, side, lidx, 1, move))
                st += [s_meas1, s_macc]
                return st

            # rep 0 runs SOLO at the head (the PE is DMA-bound there anyway);
            # then three chains cover reps 1..24, round-robin one stage at a
            # time so every engine sees a steady interleave.
            chains = [
                [emit_rep(rr, ch) for rr in range(1 + ch, RL, 3)]
                for ch in range(3)
            ]
            solo = emit_rep(0, 0)
            for s in solo:
                s()
            queues = [[s for rep in chain for s in rep] for chain in chains]
            pos = [0, 0, 0]
            while any(pos[i] < len(queues[i]) for i in range(3)):
                for i in range(3):
                    if pos[i] < len(queues[i]):
                        queues[i][pos[i]]()
                        pos[i] += 1

            # final: copy accumulator to SBUF, DMA out
            outs = sp.tile([24, 32], F32, name="outs", tag="outs")
            nc.vector.tensor_copy(outs, macc)
            nc.sync.dma_start(d_out.ap(), outs)

    nc.compile()
    return nc


# ---------------- public entry ----------------
_CACHE = {}


def _get_nc():
    if "nc" not in _CACHE:
        _CACHE["nc"] = build_nc()
    return _CACHE["nc"]


def run(inputs, trace=False):
    shared, percore = host_prep(inputs["data_angles"], inputs["params"],
                                inputs["noise_choices"])
    nc = _get_nc()
    in_maps = []
    for c in range(NCORES):
        m = dict(shared)
        m.update(percore[c])
        in_maps.append(m)
    res = bass_utils.run_bass_kernel_spmd(nc, in_maps, list(range(NCORES)),
                                          trace=trace)
    acc = np.zeros((24, 32), np.float64)
    for c in range(NCORES):
        acc += np.asarray(res.results[c]["out"], np.float64)
    return decode_output(acc), res


def kernel(**inputs):
    out, _ = run(inputs)
    return out


# revision 22
# speedup vs baseline: 1.2717x; 1.0015x over previous
"""Trainium2 Bass kernel for NoisyPQC (12-qubit noisy PQC expectation values).

Math restructure (validated vs reference in numpy):
  state index s = p*32 + f;  p = qubits 0..6 (qubit 0 = MSB of p),
  f = qubits 7..11.  state.reshape(128,32)[p,f] == state[s].
  Per trajectory r:  x = B3 D B2 D B1 D B0 psi0 with
    B0 = U0, Bl = Ul*Perm(m_{l-1})*Diag(sigma_{l-1}),
    D = (D_P (x) D_F) * C,  C[p,f] = (-1)^((p&1)*((f>>4)&1)).
  D_P/D_F fold into columns of B_l (l>=1); C applied elementwise 3x.
  Layer 0 is trajectory-independent -> host precomputes
    state1 = C * (GP0 @ psi0 @ GF0^T)  for all 16 batches.
  Device per (r): 3 layers of  phi = GP_l @ phi @ GF_l^T  (x C for l<3),
  then probs = |phi|^2, contracted with z-sign tables; final-layer noise
  becomes sign flips folded into the measurement matrices.

v4 scheme ("stationary-state" matmuls as v2, plus):
  - gate tables store [-B|A|B] (1152 cols/rep/tensor): the two moving
    pair planes [A|B] and [-B|A] are overlapping 256-col windows, so the
    HBM gate stream drops 25%.
  - every gate stage is TWO half-stages of 512 psum cols (one bank);
    per-chain mm tag with bufs=2 decouples the PE from the PSUM->SBUF
    moves: the PE runs ahead into the other bank while a move drains.
  - THREE interleaved chains (reps 1..24) + rep 0 solo at the head where
    the PE is DMA-bound anyway.  Each chain's meas1/transpose PSUM lives
    inside its own mm-bank rotation -> 7 banks total, no cross-chain
    PSUM contention.
  - moves: h0 copy on ACT / h1 copy on DVE; xC multiplies on DVE (h0+h1
    via the C table); final-layer moves ARE the ACT squares (PSUM->SBUF
    with Square), no separate square pass, contiguous (not ri-major) -
    meas1 instead sums ri via two strided-moving matmuls per half.

Layouts (b = 4*b_hi + b_lo):
  N: [p=128 part, col = b_hi*256 + ri*128 + b_lo*32 + f]   (ri: 0=Re,1=Im)
  T: [b_lo*32+f = 128 part, col = b_hi*256 + ri*128 + p]
P-stage (N->T), per b_hi: mm pairA=[GPr^T|GPi^T] then pairB=[-GPi^T|GPr^T]
accumulated.  F-stage (T->N) same with kron(I4, .) planes.  All mms
stream 256 cols => full-speed f32r (1 cyc/row).

Sharding: 200 trajectories = 8 cores x 25. Each core accumulates its 25
trajectories' (sign-flipped) measurement vectors into one PSUM bank via a
single open accumulation group; host sums the 8 [24,32] outputs and /200.
"""

import sys

for _p in ("/opt/trn_rl_repo",):
    if _p not in sys.path:
        sys.path.insert(0, _p)

import numpy as np

from concourse import bacc, bass_utils, mybir
import concourse.tile as tile

# ---------------- problem constants (hardcoded per contract) ----------------
NQ = 12
NL = 4
REPS = 200
BATCH = 16
NCORES = 8
RL = REPS // NCORES  # 25 reps per core
DP, DF = 128, 32  # dim of p-side (qubits 0..6) and f-side (qubits 7..11)
GW = 384  # gate cols per layer: [-B|A|B]; pairB = cols 0:256, pairA = 128:384

F32 = mybir.dt.float32
F32R = mybir.dt.float32r
FP16 = mybir.dt.float16


# ---------------- host-side math ----------------
def _rx(t):
    c, s = np.cos(0.5 * t), -1j * np.sin(0.5 * t)
    return np.array([[c, s], [s, c]], np.complex64)


def _rz(t):
    return np.array([[np.exp(-0.5j * t), 0], [0, np.exp(0.5j * t)]], np.complex64)


def _kron_list(mats):
    out = mats[0]
    for m in mats[1:]:
        out = np.kron(out, m)
    return out


def _tables():
    p = np.arange(DP)
    f = np.arange(DF)
    dP = np.ones(DP)
    for j in range(6):
        dP *= np.where(((p >> (6 - j)) & 1) & ((p >> (5 - j)) & 1), -1.0, 1.0)
    dF = np.ones(DF)
    for k in range(4):
        dF *= np.where(((f >> (4 - k)) & 1) & ((f >> (3 - k)) & 1), -1.0, 1.0)
    C = np.where(((p[:, None] & 1) & ((f[None, :] >> 4) & 1)) == 1, -1.0, 1.0)
    zP = 1.0 - 2.0 * ((p[None, :] >> (6 - np.arange(7)[:, None])) & 1)  # [7,128]
    zF = 1.0 - 2.0 * ((f[None, :] >> (4 - np.arange(5)[:, None])) & 1)  # [5,32]
    return dP, dF, C, zP, zF


def host_prep(data_angles, params, noise):
    """Build all device arrays. Returns (shared dict, per-core list of dicts)."""
    da = np.asarray(data_angles, np.float64)
    pa = np.asarray(params, np.float64)
    nz = np.asarray(noise)
    dPt, dFt, C, zP, zF = _tables()

    # base per-qubit gates u[l][q] = Rx(params[l,q,1]) @ Rz(params[l,q,0])
    u = [[_rx(pa[l, q, 1]) @ _rz(pa[l, q, 0]) for q in range(NQ)] for l in range(NL)]

    # --- state after layer 0 (incl. C), identical for every trajectory ---
    va = np.stack([np.cos(0.5 * da), -1j * np.sin(0.5 * da)], -1)  # [B,12,2]
    GP0 = _kron_list([u[0][q] for q in range(7)])        # [128,128]
    GF0 = _kron_list([u[0][q] for q in range(7, NQ)])    # [32,32]
    s_re = np.empty((DP, BATCH * DF), np.float32)
    s_im = np.empty((DP, BATCH * DF), np.float32)
    for b in range(BATCH):
        vPr = _kron_list([va[b, q].astype(np.complex128) for q in range(7)])
        vFr = _kron_list([va[b, q].astype(np.complex128) for q in range(7, NQ)])
        phi = (GP0 @ np.outer(vPr, vFr) @ GF0.T) * C
        s_re[:, b * DF:(b + 1) * DF] = phi.real
        s_im[:, b * DF:(b + 1) * DF] = phi.imag
    # N layout: col = b_hi*256 + ri*128 + b_lo*32 + f
    state1 = np.empty((DP, 1024), np.float32)
    for bh in range(4):
        state1[:, bh * 256:bh * 256 + 128] = s_re[:, bh * 128:bh * 128 + 128]
        state1[:, bh * 256 + 128:bh * 256 + 256] = s_im[:, bh * 128:bh * 128 + 128]

    # --- per-core moving gate tables + measurement tables ---
    # per layer block (GW=384 cols): [-B | A | B] with A = Re-plane, B =
    # Im-plane; pairA=[A|B] = cols 128:384, pairB=[-B|A] = cols 0:256.
    eye4 = np.eye(4)
    percore = []
    for c in range(NCORES):
        gp = np.empty((RL, DP, 3 * GW), np.float32)
        gf = np.empty((RL, 32, 9 * DF), np.float32)
        m1 = np.empty((DP, RL * 24), np.float32)
        m2 = np.empty((DP, RL * 8), np.float32)
        for rl in range(RL):
            r = c * RL + rl
            for lidx, l in enumerate((1, 2, 3)):
                facs = []
                for q in range(NQ):
                    g = u[l][q]
                    ch = int(nz[r, l - 1, q])
                    if ch in (1, 2):
                        g = g[:, ::-1]
                    if ch in (2, 3):
                        g = g * np.array([1.0, -1.0])[None, :]
                    facs.append(g)
                GP = _kron_list(facs[:7]) * dPt[None, :]
                GF = _kron_list(facs[7:]) * dFt[None, :]
                o = lidx * GW
                gp[rl, :, o:o + 128] = -GP.imag.T
                gp[rl, :, o + 128:o + 256] = GP.real.T
                gp[rl, :, o + 256:o + 384] = GP.imag.T
                of = lidx * 3 * DF
                gf[rl, :, of:of + DF] = -GF.imag.T
                gf[rl, :, of + DF:of + 2 * DF] = GF.real.T
                gf[rl, :, of + 2 * DF:of + 3 * DF] = GF.imag.T
            m3 = nz[r, NL - 1]  # [12]
            flip = np.where((m3 == 1) | (m3 == 2), -1.0, 1.0)
            M1 = np.ones((DF, 6))
            M1[:, :5] = (zF * flip[7:, None]).T  # [32,5]
            m1[:, rl * 24:(rl + 1) * 24] = np.kron(eye4, M1)
            M2 = np.ones((DP, 8), np.float64)
            M2[:, :7] = (zP * flip[:7, None]).T
            m2[:, rl * 8:(rl + 1) * 8] = M2
        percore.append({"gp": gp, "gf": gf, "meas1": m1, "meas2": m2})

    shared = {
        "state1": np.ascontiguousarray(state1),
        "ctabN": np.ascontiguousarray(np.tile(C, (1, 32)).astype(np.float32)),
        "ident": np.eye(8, dtype=np.float32),
    }
    for d in percore:
        for k in list(d):
            d[k] = np.ascontiguousarray(d[k], np.float16)
    for k in ("state1", "ident"):
        shared[k] = np.ascontiguousarray(shared[k], np.float16)
    return shared, percore


def decode_output(acc):
    """acc: [24,32] summed over cores. Returns [16,12] float32."""
    out = np.empty((BATCH, NQ), np.float64)
    for bh in range(4):
        for bl in range(4):
            b = 4 * bh + bl
            for q in range(7):
                out[b, q] = acc[bl * 6 + 5, bh * 8 + q]
            for j in range(5):
                out[b, 7 + j] = acc[bl * 6 + j, bh * 8 + 7]
    return (out / REPS).astype(np.float32)


# ---------------- device kernel ----------------
def build_nc(dt=FP16, debug=False):
    """Build + compile the per-core Bass program (same for all cores)."""
    nc = bacc.Bacc("TRN2", target_bir_lowering=False, debug=debug,
                   num_devices=NCORES)
    # fp16 everywhere a matmul operand lives: same 1 cyc/row PE rate as
    # f32r but half the DMA/SBUF traffic and far lower PE power (less
    # DVFS throttling).  PSUM stays f32.
    d_state1 = nc.dram_tensor("state1", (DP, 1024), dt, kind="ExternalInput")
    d_gp = nc.dram_tensor("gp", (RL, DP, 3 * GW), dt, kind="ExternalInput")
    d_gf = nc.dram_tensor("gf", (RL, 32, 9 * DF), dt, kind="ExternalInput")
    d_m1 = nc.dram_tensor("meas1", (DP, RL * 24), dt, kind="ExternalInput")
    d_m2 = nc.dram_tensor("meas2", (DP, RL * 8), dt, kind="ExternalInput")
    d_ctn = nc.dram_tensor("ctabN", (DP, 1024), F32, kind="ExternalInput")
    d_id = nc.dram_tensor("ident", (8, 8), dt, kind="ExternalInput")
    d_out = nc.dram_tensor("out", (24, 32), F32, kind="ExternalOutput")

    MUL = mybir.AluOpType.mult
    ADD = mybir.AluOpType.add
    SQ = mybir.ActivationFunctionType.Square

    with tile.TileContext(nc) as tc:
        from contextlib import ExitStack
        with ExitStack() as ex:
            cp = ex.enter_context(tc.tile_pool(name="const", bufs=1))
            sp = ex.enter_context(tc.tile_pool(name="work", bufs=1))
            pp = ex.enter_context(tc.tile_pool(name="ps", bufs=1, space="PSUM"))

            # constants -> SBUF once
            c_state1 = cp.tile([DP, 1024], dt, name="state1", tag="state1")
            c_ctn = cp.tile([DP, 1024], F32, name="ctn", tag="ctn")
            c_id = cp.tile([8, 8], dt, name="ident", tag="ident")
            c_m1 = cp.tile([DP, RL * 24], dt, name="m1", tag="m1")
            c_m2 = cp.tile([DP, RL * 8], dt, name="m2", tag="m2")
            # consts ride on the ACT queue's DGE ring so the sync+gpsimd
            # rings are free for the per-rep gate streams; need-order.
            nc.scalar.dma_start(c_state1[:, 0:512], d_state1.ap()[:, 0:512])
            nc.scalar.dma_start(c_state1[:, 512:1024], d_state1.ap()[:, 512:1024])
            nc.scalar.dma_start(c_ctn, d_ctn.ap())
            nc.scalar.dma_start(c_m2, d_m2.ap())
            nc.scalar.dma_start(c_m1, d_m1.ap())
            nc.scalar.dma_start(c_id, d_id.ap())

            macc = pp.tile([24, 32], F32, name="macc", tag="macc")
            macc_n = [0]

            # zero the gf double-buffers once: per-rep DMAs only scatter the
            # 4 diagonal kron blocks.  Two dummy allocations per tag walk
            # each rotation exactly one full cycle.
            for zch in range(3):
                for _ in range(2):
                    z = sp.tile([DP, 3 * GW], dt, name="gfz", tag="gf" + str(zch),
                                bufs=2)
                    nc.vector.memset(z.bitcast(mybir.dt.uint32), 0)

            def emit_rep(r, ch):
                """Return list of 16 stage closures for trajectory r, chain ch."""
                t = {}
                g = f"{ch}"

                def s_dma():
                    t["gp"] = sp.tile([DP, 3 * GW], dt, name="gp", tag="gp" + g, bufs=2)
                    t["gf"] = sp.tile([DP, 3 * GW], dt, name="gf", tag="gf" + g, bufs=2)
                    if r == 0:
                        # first rep: per-layer chunks so the first stage only
                        # waits one small transfer
                        for l3 in range(3):
                            cs = slice(l3 * GW, (l3 + 1) * GW)
                            eng = nc.sync if l3 < 2 else nc.gpsimd
                            eng.dma_start(t["gp"][:, cs], d_gp.ap()[r][:, cs])
                    else:
                        # one 1152-col transfer: 2.3KB per-partition lines
                        # (the DGE efficiency threshold is 2KB); alternate
                        # rings to balance with the gf scatters
                        eng = nc.sync if r % 2 else nc.gpsimd
                        eng.dma_start(t["gp"], d_gp.ap()[r])
                    # gf planes are kron(I4, .): scatter only the 32-row
                    # content into the 4 diagonal blocks; the off-diagonal
                    # zeros were memset once at startup.
                    gft = t["gf"]
                    srf = d_gf.ap()[r]
                    APc, VP = type(gft), type(gft.ap)
                    for bl in range(4):
                        dst = APc(tensor=gft.tensor,
                                  offset=gft.offset + bl * 32 * 1152 + bl * 32,
                                  ap=VP([[1152, 32], [128, 9], [1, 32]]))
                        srcb = APc(tensor=srf.tensor, offset=srf.offset,
                                   ap=VP([[288, 32], [32, 9], [1, 32]]))
                        nc.gpsimd.dma_start(dst, srcb)

                def new_mm():
                    return pp.tile([DP, 512], F32, name="mm", tag="mm" + g, bufs=2)

                def mk_half(stat_key, side, lidx, half, move):
                    """One half-stage: 4 matmuls (b_hi pair 2*half, 2*half+1)
                    into a single-bank [128,512] psum tile, then this half's
                    PSUM->SBUF move.  Stationary = state cols of the global
                    b_hi block; moving = the 256-col gate pair windows."""
                    def s():
                        mm = new_mm()
                        stat = c_state1 if stat_key is None else t[stat_key]
                        mov = t["gp"] if side == "P" else t["gf"]
                        base = lidx * GW
                        movA = mov[:, base + 128:base + 384]
                        movB = mov[:, base:base + 256]
                        for j, bh in enumerate((2 * half, 2 * half + 1)):
                            o, so = j * 256, bh * 256
                            nc.tensor.matmul(mm[:, o:o + 256],
                                             stat[:, so:so + 128], movA,
                                             start=True, stop=False)
                            nc.tensor.matmul(mm[:, o:o + 256],
                                             stat[:, so + 128:so + 256], movB,
                                             start=False, stop=True)
                        move(mm, half)
                    return s

                def mv_copy(key):
                    # h0 on ACT, h1 on DVE: each a single 512-col move with a
                    # multi-half-stage window before the bank is needed again.
                    def m(mm, half):
                        if half == 0:
                            t[key] = sp.tile([DP, 1024], dt, name="tmp",
                                             tag="tmp" + g, bufs=2)
                            nc.scalar.copy(t[key][:, 0:512], mm)
                        else:
                            nc.vector.tensor_copy(t[key][:, 512:1024], mm)
                    return m

                def mv_cmul(key):
                    # x C on DVE via the C table (ACT cannot tensor_tensor)
                    def m(mm, half):
                        if half == 0:
                            t[key] = sp.tile([DP, 1024], dt, name="st",
                                             tag="st" + g, bufs=2)
                        cs = slice(half * 512, (half + 1) * 512)
                        nc.vector.tensor_tensor(t[key][:, cs], mm, c_ctn[:, cs], MUL)
                    return m

                def mv_square(mm, half):
                    # final layer: the move IS the square (contiguous layout,
                    # cols (bh, ri, b_lo, f) for bh pair of this half), then
                    # one strided DVE add pre-sums ri: sqs = re^2 + im^2,
                    # cols (bh, b_lo, f), halving the meas matmul count.
                    key = "sqA" if half == 0 else "sqB"
                    t[key] = sp.tile([DP, 512], dt, name=key, tag=key + g, bufs=2)
                    sq = t[key]
                    nc.scalar.activation(sq[:, 0:512], mm, SQ)
                    ks = key + "s"
                    t[ks] = sp.tile([DP, 256], dt, name=ks, tag=ks + g, bufs=2)
                    APc, VP = type(sq), type(sq.ap)
                    in0 = APc(tensor=sq.tensor, offset=sq.offset,
                              ap=VP([[512, DP], [256, 2], [1, 128]]))
                    in1 = APc(tensor=sq.tensor, offset=sq.offset + 128,
                              ap=VP([[512, DP], [256, 2], [1, 128]]))
                    nc.vector.tensor_tensor(t[ks], in0, in1, ADD)

                def s_meas1():
                    # p-contraction with the OUTPUT TRANSPOSED: the ri-summed
                    # squares are the stationary operand, the per-rep m2
                    # column block the moving one, so out partitions = sq
                    # columns = (bl,f) and out cols = q.  fp16 matmuls run
                    # 1 cyc/row at any width: 4 narrow matmuls replace
                    # meas1 + 4 PE transposes (and their mode-switch drains).
                    mp = new_mm()
                    trm = mp[:, 0:32]
                    mov = c_m2[:, r * 8:(r + 1) * 8]
                    for bh in range(4):
                        sqs = t["sqAs"] if bh < 2 else t["sqBs"]
                        nc.tensor.matmul(trm[:, bh * 8:bh * 8 + 8],
                                         sqs[:, (bh % 2) * 128:(bh % 2) * 128 + 128],
                                         mov, start=True, stop=True)
                    t["trs"] = sp.tile([DP, 32], dt, name="trs", tag="trs" + g,
                                       bufs=2)
                    nc.vector.tensor_copy(t["trs"], trm)

                def s_macc():
                    macc_n[0] += 1
                    nc.tensor.matmul(macc, c_m1[:, r * 24:(r + 1) * 24], t["trs"],
                                     start=(macc_n[0] == 1),
                                     stop=(macc_n[0] == RL))

                st = [s_dma]
                plan = [(None, "P", 0, mv_copy("t1")),
                        ("t1", "F", 0, mv_cmul("s1")),
                        ("s1", "P", 1, mv_copy("t2")),
                        ("t2", "F", 1, mv_cmul("s2")),
                        ("s2", "P", 2, mv_copy("t3")),
                        ("t3", "F", 2, mv_square)]
                for stat_key, side, lidx, move in plan:
                    st.append(mk_half(stat_key, side, lidx, 0, move))
                    st.append(mk_half(stat_key, side, lidx, 1, move))
                st += [s_meas1, s_macc]
                return st

            # rep 0 runs SOLO at the head (the PE is DMA-bound there anyway);
            # then three chains cover reps 1..24, round-robin one stage at a
            # time so every engine sees a steady interleave.
            chains = [
                [emit_rep(rr, ch) for rr in range(1 + ch, RL, 3)]
                for ch in range(3)
            ]
            solo = emit_rep(0, 0)
            for s in solo:
                s()
            queues = [[s for rep in chain for s in rep] for chain in chains]
            pos = [0, 0, 0]
            while any(pos[i] < len(queues[i]) for i in range(3)):
                for i in range(3):
                    if pos[i] < len(queues[i]):
                        queues[i][pos[i]]()
                        pos[i] += 1

            # final: copy accumulator to SBUF, DMA out
            outs = sp.tile([24, 32], F32, name="outs", tag="outs")
            nc.vector.tensor_copy(outs, macc)
            nc.sync.dma_start(d_out.ap(), outs)

    nc.compile()
    return nc


# ---------------- public entry ----------------
_CACHE = {}


def _get_nc():
    if "nc" not in _CACHE:
        _CACHE["nc"] = build_nc()
    return _CACHE["nc"]


def run(inputs, trace=False):
    shared, percore = host_prep(inputs["data_angles"], inputs["params"],
                                inputs["noise_choices"])
    nc = _get_nc()
    in_maps = []
    for c in range(NCORES):
        m = dict(shared)
        m.update(percore[c])
        in_maps.append(m)
    res = bass_utils.run_bass_kernel_spmd(nc, in_maps, list(range(NCORES)),
                                          trace=trace)
    acc = np.zeros((24, 32), np.float64)
    for c in range(NCORES):
        acc += np.asarray(res.results[c]["out"], np.float64)
    return decode_output(acc), res


def kernel(**inputs):
    out, _ = run(inputs)
    return out


# revision 23
# speedup vs baseline: 1.2974x; 1.0202x over previous
"""Trainium2 Bass kernel for NoisyPQC (12-qubit noisy PQC expectation values).

Math restructure (validated vs reference in numpy):
  state index s = p*32 + f;  p = qubits 0..6 (qubit 0 = MSB of p),
  f = qubits 7..11.  state.reshape(128,32)[p,f] == state[s].
  Per trajectory r:  x = B3 D B2 D B1 D B0 psi0 with
    B0 = U0, Bl = Ul*Perm(m_{l-1})*Diag(sigma_{l-1}),
    D = (D_P (x) D_F) * C,  C[p,f] = (-1)^((p&1)*((f>>4)&1)).
  D_P/D_F fold into columns of B_l (l>=1); C applied elementwise 3x.
  Layer 0 is trajectory-independent -> host precomputes
    state1 = C * (GP0 @ psi0 @ GF0^T)  for all 16 batches.
  Device per (r): 3 layers of  phi = GP_l @ phi @ GF_l^T  (x C for l<3),
  then probs = |phi|^2, contracted with z-sign tables; final-layer noise
  becomes sign flips folded into the measurement matrices.

v4 scheme ("stationary-state" matmuls as v2, plus):
  - gate tables store [-B|A|B] (1152 cols/rep/tensor): the two moving
    pair planes [A|B] and [-B|A] are overlapping 256-col windows, so the
    HBM gate stream drops 25%.
  - every gate stage is TWO half-stages of 512 psum cols (one bank);
    per-chain mm tag with bufs=2 decouples the PE from the PSUM->SBUF
    moves: the PE runs ahead into the other bank while a move drains.
  - THREE interleaved chains (reps 1..24) + rep 0 solo at the head where
    the PE is DMA-bound anyway.  Each chain's meas1/transpose PSUM lives
    inside its own mm-bank rotation -> 7 banks total, no cross-chain
    PSUM contention.
  - moves: h0 copy on ACT / h1 copy on DVE; xC multiplies on DVE (h0+h1
    via the C table); final-layer moves ARE the ACT squares (PSUM->SBUF
    with Square), no separate square pass, contiguous (not ri-major) -
    meas1 instead sums ri via two strided-moving matmuls per half.

Layouts (b = 4*b_hi + b_lo):
  N: [p=128 part, col = b_hi*256 + ri*128 + b_lo*32 + f]   (ri: 0=Re,1=Im)
  T: [b_lo*32+f = 128 part, col = b_hi*256 + ri*128 + p]
P-stage (N->T), per b_hi: mm pairA=[GPr^T|GPi^T] then pairB=[-GPi^T|GPr^T]
accumulated.  F-stage (T->N) same with kron(I4, .) planes.  All mms
stream 256 cols => full-speed f32r (1 cyc/row).

Sharding: 200 trajectories = 8 cores x 25. Each core accumulates its 25
trajectories' (sign-flipped) measurement vectors into one PSUM bank via a
single open accumulation group; host sums the 8 [24,32] outputs and /200.
"""

import sys

for _p in ("/opt/trn_rl_repo",):
    if _p not in sys.path:
        sys.path.insert(0, _p)

import numpy as np

from concourse import bacc, bass_utils, mybir
import concourse.tile as tile

# ---------------- problem constants (hardcoded per contract) ----------------
NQ = 12
NL = 4
REPS = 200
BATCH = 16
NCORES = 8
RL = REPS // NCORES  # 25 reps per core
DP, DF = 128, 32  # dim of p-side (qubits 0..6) and f-side (qubits 7..11)
GW = 384  # gate cols per layer: [-B|A|B]; pairB = cols 0:256, pairA = 128:384

F32 = mybir.dt.float32
F32R = mybir.dt.float32r
FP16 = mybir.dt.float16


# ---------------- host-side math ----------------
def _rx(t):
    c, s = np.cos(0.5 * t), -1j * np.sin(0.5 * t)
    return np.array([[c, s], [s, c]], np.complex64)


def _rz(t):
    return np.array([[np.exp(-0.5j * t), 0], [0, np.exp(0.5j * t)]], np.complex64)


def _kron_list(mats):
    out = mats[0]
    for m in mats[1:]:
        out = np.kron(out, m)
    return out


def _tables():
    p = np.arange(DP)
    f = np.arange(DF)
    dP = np.ones(DP)
    for j in range(6):
        dP *= np.where(((p >> (6 - j)) & 1) & ((p >> (5 - j)) & 1), -1.0, 1.0)
    dF = np.ones(DF)
    for k in range(4):
        dF *= np.where(((f >> (4 - k)) & 1) & ((f >> (3 - k)) & 1), -1.0, 1.0)
    C = np.where(((p[:, None] & 1) & ((f[None, :] >> 4) & 1)) == 1, -1.0, 1.0)
    zP = 1.0 - 2.0 * ((p[None, :] >> (6 - np.arange(7)[:, None])) & 1)  # [7,128]
    zF = 1.0 - 2.0 * ((f[None, :] >> (4 - np.arange(5)[:, None])) & 1)  # [5,32]
    return dP, dF, C, zP, zF


def host_prep(data_angles, params, noise):
    """Build all device arrays. Returns (shared dict, per-core list of dicts)."""
    da = np.asarray(data_angles, np.float64)
    pa = np.asarray(params, np.float64)
    nz = np.asarray(noise)
    dPt, dFt, C, zP, zF = _tables()

    # base per-qubit gates u[l][q] = Rx(params[l,q,1]) @ Rz(params[l,q,0])
    u = [[_rx(pa[l, q, 1]) @ _rz(pa[l, q, 0]) for q in range(NQ)] for l in range(NL)]

    # --- state after layer 0 (incl. C), identical for every trajectory ---
    va = np.stack([np.cos(0.5 * da), -1j * np.sin(0.5 * da)], -1)  # [B,12,2]
    GP0 = _kron_list([u[0][q] for q in range(7)])        # [128,128]
    GF0 = _kron_list([u[0][q] for q in range(7, NQ)])    # [32,32]
    s_re = np.empty((DP, BATCH * DF), np.float32)
    s_im = np.empty((DP, BATCH * DF), np.float32)
    for b in range(BATCH):
        vPr = _kron_list([va[b, q].astype(np.complex128) for q in range(7)])
        vFr = _kron_list([va[b, q].astype(np.complex128) for q in range(7, NQ)])
        phi = (GP0 @ np.outer(vPr, vFr) @ GF0.T) * C
        s_re[:, b * DF:(b + 1) * DF] = phi.real
        s_im[:, b * DF:(b + 1) * DF] = phi.imag
    # N layout: col = b_hi*256 + ri*128 + b_lo*32 + f
    state1 = np.empty((DP, 1024), np.float32)
    for bh in range(4):
        state1[:, bh * 256:bh * 256 + 128] = s_re[:, bh * 128:bh * 128 + 128]
        state1[:, bh * 256 + 128:bh * 256 + 256] = s_im[:, bh * 128:bh * 128 + 128]

    # --- per-core moving gate tables + measurement tables ---
    # per layer block (GW=384 cols): [-B | A | B] with A = Re-plane, B =
    # Im-plane; pairA=[A|B] = cols 128:384, pairB=[-B|A] = cols 0:256.
    eye4 = np.eye(4)
    percore = []
    for c in range(NCORES):
        gp = np.empty((RL, DP, 3 * GW), np.float32)
        gf = np.empty((RL, 32, 9 * DF), np.float32)
        m1 = np.empty((DP, RL * 24), np.float32)
        m2 = np.empty((DP, RL * 8), np.float32)
        for rl in range(RL):
            r = c * RL + rl
            for lidx, l in enumerate((1, 2, 3)):
                facs = []
                for q in range(NQ):
                    g = u[l][q]
                    ch = int(nz[r, l - 1, q])
                    if ch in (1, 2):
                        g = g[:, ::-1]
                    if ch in (2, 3):
                        g = g * np.array([1.0, -1.0])[None, :]
                    facs.append(g)
                GP = _kron_list(facs[:7]) * dPt[None, :]
                GF = _kron_list(facs[7:]) * dFt[None, :]
                o = lidx * GW
                gp[rl, :, o:o + 128] = -GP.imag.T
                gp[rl, :, o + 128:o + 256] = GP.real.T
                gp[rl, :, o + 256:o + 384] = GP.imag.T
                of = lidx * 3 * DF
                gf[rl, :, of:of + DF] = -GF.imag.T
                gf[rl, :, of + DF:of + 2 * DF] = GF.real.T
                gf[rl, :, of + 2 * DF:of + 3 * DF] = GF.imag.T
            m3 = nz[r, NL - 1]  # [12]
            flip = np.where((m3 == 1) | (m3 == 2), -1.0, 1.0)
            M1 = np.ones((DF, 6))
            M1[:, :5] = (zF * flip[7:, None]).T  # [32,5]
            m1[:, rl * 24:(rl + 1) * 24] = np.kron(eye4, M1)
            M2 = np.ones((DP, 8), np.float64)
            M2[:, :7] = (zP * flip[:7, None]).T
            m2[:, rl * 8:(rl + 1) * 8] = M2
        percore.append({"gp": gp, "gf": gf, "meas1": m1, "meas2": m2})

    shared = {
        "state1": np.ascontiguousarray(state1),
        "ctabN": np.ascontiguousarray(np.tile(C, (1, 32)).astype(np.float32)),
        "ident": np.eye(8, dtype=np.float32),
    }
    for d in percore:
        for k in list(d):
            d[k] = np.ascontiguousarray(d[k], np.float16)
    for k in ("state1", "ident"):
        shared[k] = np.ascontiguousarray(shared[k], np.float16)
    return shared, percore


def decode_output(acc):
    """acc: [24,32] summed over cores. Returns [16,12] float32."""
    out = np.empty((BATCH, NQ), np.float64)
    for bh in range(4):
        for bl in range(4):
            b = 4 * bh + bl
            for q in range(7):
                out[b, q] = acc[bl * 6 + 5, bh * 8 + q]
            for j in range(5):
                out[b, 7 + j] = acc[bl * 6 + j, bh * 8 + 7]
    return (out / REPS).astype(np.float32)


# ---------------- device kernel ----------------
def build_nc(dt=FP16, debug=False):
    """Build + compile the per-core Bass program (same for all cores)."""
    nc = bacc.Bacc("TRN2", target_bir_lowering=False, debug=debug,
                   num_devices=NCORES)
    # fp16 everywhere a matmul operand lives: same 1 cyc/row PE rate as
    # f32r but half the DMA/SBUF traffic and far lower PE power (less
    # DVFS throttling).  PSUM stays f32.
    d_state1 = nc.dram_tensor("state1", (DP, 1024), dt, kind="ExternalInput")
    d_gp = nc.dram_tensor("gp", (RL, DP, 3 * GW), dt, kind="ExternalInput")
    d_gf = nc.dram_tensor("gf", (RL, 32, 9 * DF), dt, kind="ExternalInput")
    d_m1 = nc.dram_tensor("meas1", (DP, RL * 24), dt, kind="ExternalInput")
    d_m2 = nc.dram_tensor("meas2", (DP, RL * 8), dt, kind="ExternalInput")
    d_ctn = nc.dram_tensor("ctabN", (DP, 1024), F32, kind="ExternalInput")
    d_id = nc.dram_tensor("ident", (8, 8), dt, kind="ExternalInput")
    d_out = nc.dram_tensor("out", (24, 32), F32, kind="ExternalOutput")

    MUL = mybir.AluOpType.mult
    ADD = mybir.AluOpType.add
    SQ = mybir.ActivationFunctionType.Square

    with tile.TileContext(nc) as tc:
        from contextlib import ExitStack
        with ExitStack() as ex:
            cp = ex.enter_context(tc.tile_pool(name="const", bufs=1))
            sp = ex.enter_context(tc.tile_pool(name="work", bufs=1))
            pp = ex.enter_context(tc.tile_pool(name="ps", bufs=1, space="PSUM"))

            # constants -> SBUF once
            c_state1 = cp.tile([DP, 1024], dt, name="state1", tag="state1")
            c_ctn = cp.tile([DP, 1024], F32, name="ctn", tag="ctn")
            c_id = cp.tile([8, 8], dt, name="ident", tag="ident")
            c_m1 = cp.tile([DP, RL * 24], dt, name="m1", tag="m1")
            c_m2 = cp.tile([DP, RL * 8], dt, name="m2", tag="m2")
            # consts ride on the ACT queue's DGE ring so the sync+gpsimd
            # rings are free for the per-rep gate streams; need-order.
            nc.scalar.dma_start(c_state1[:, 0:512], d_state1.ap()[:, 0:512])
            nc.scalar.dma_start(c_state1[:, 512:1024], d_state1.ap()[:, 512:1024])
            nc.scalar.dma_start(c_ctn, d_ctn.ap())
            nc.scalar.dma_start(c_m2, d_m2.ap())
            nc.scalar.dma_start(c_m1, d_m1.ap())
            nc.scalar.dma_start(c_id, d_id.ap())

            macc = pp.tile([24, 32], F32, name="macc", tag="macc")
            macc_n = [0]

            # zero the gf double-buffers once: per-rep DMAs only scatter the
            # 4 diagonal kron blocks.  Two dummy allocations per tag walk
            # each rotation exactly one full cycle.
            for zch in range(3):
                for _ in range(2):
                    z = sp.tile([DP, 3 * GW], dt, name="gfz", tag="gf" + str(zch),
                                bufs=2)
                    nc.vector.memset(z.bitcast(mybir.dt.uint32), 0)

            def emit_rep(r, ch):
                """Return list of 16 stage closures for trajectory r, chain ch."""
                t = {}
                g = f"{ch}"

                def s_dma():
                    t["gp"] = sp.tile([DP, 3 * GW], dt, name="gp", tag="gp" + g, bufs=2)
                    t["gf"] = sp.tile([DP, 3 * GW], dt, name="gf", tag="gf" + g, bufs=2)
                    # gf planes are kron(I4, .): scatter only the 32-row
                    # content into the 4 diagonal blocks; the off-diagonal
                    # zeros were memset once at startup.  Scatters go FIRST:
                    # a rep's gf (F stage, layer 0) is needed three stages
                    # before its gp layer 2.
                    gft = t["gf"]
                    srf = d_gf.ap()[r]
                    APc, VP = type(gft), type(gft.ap)
                    for bl in range(4):
                        dst = APc(tensor=gft.tensor,
                                  offset=gft.offset + bl * 32 * 1152 + bl * 32,
                                  ap=VP([[1152, 32], [128, 9], [1, 32]]))
                        srcb = APc(tensor=srf.tensor, offset=srf.offset,
                                   ap=VP([[288, 32], [32, 9], [1, 32]]))
                        nc.gpsimd.dma_start(dst, srcb)
                    if r <= 6:
                        # head reps: per-layer chunks so the first stages
                        # only wait small transfers while the rings ramp
                        for l3 in range(3):
                            cs = slice(l3 * GW, (l3 + 1) * GW)
                            eng = nc.sync if l3 < 2 else nc.gpsimd
                            eng.dma_start(t["gp"][:, cs], d_gp.ap()[r][:, cs])
                    else:
                        # one 1152-col transfer: 2.3KB per-partition lines
                        # (the DGE efficiency threshold is 2KB); alternate
                        # rings to balance with the gf scatters
                        eng = nc.sync if r % 2 else nc.gpsimd
                        eng.dma_start(t["gp"], d_gp.ap()[r])

                def new_mm():
                    return pp.tile([DP, 512], F32, name="mm", tag="mm" + g, bufs=2)

                def mk_half(stat_key, side, lidx, half, move):
                    """One half-stage: 4 matmuls (b_hi pair 2*half, 2*half+1)
                    into a single-bank [128,512] psum tile, then this half's
                    PSUM->SBUF move.  Stationary = state cols of the global
                    b_hi block; moving = the 256-col gate pair windows."""
                    def s():
                        mm = new_mm()
                        stat = c_state1 if stat_key is None else t[stat_key]
                        mov = t["gp"] if side == "P" else t["gf"]
                        base = lidx * GW
                        movA = mov[:, base + 128:base + 384]
                        movB = mov[:, base:base + 256]
                        for j, bh in enumerate((2 * half, 2 * half + 1)):
                            o, so = j * 256, bh * 256
                            nc.tensor.matmul(mm[:, o:o + 256],
                                             stat[:, so:so + 128], movA,
                                             start=True, stop=False)
                            nc.tensor.matmul(mm[:, o:o + 256],
                                             stat[:, so + 128:so + 256], movB,
                                             start=False, stop=True)
                        move(mm, half)
                    return s

                def mv_copy(key):
                    # h0 on ACT, h1 on DVE: each a single 512-col move with a
                    # multi-half-stage window before the bank is needed again.
                    def m(mm, half):
                        if half == 0:
                            t[key] = sp.tile([DP, 1024], dt, name="tmp",
                                             tag="tmp" + g, bufs=2)
                            nc.scalar.copy(t[key][:, 0:512], mm)
                        else:
                            nc.vector.tensor_copy(t[key][:, 512:1024], mm)
                    return m

                def mv_cmul(key):
                    # x C on DVE via the C table (ACT cannot tensor_tensor)
                    def m(mm, half):
                        if half == 0:
                            t[key] = sp.tile([DP, 1024], dt, name="st",
                                             tag="st" + g, bufs=2)
                        cs = slice(half * 512, (half + 1) * 512)
                        nc.vector.tensor_tensor(t[key][:, cs], mm, c_ctn[:, cs], MUL)
                    return m

                def mv_square(mm, half):
                    # final layer: the move IS the square (contiguous layout,
                    # cols (bh, ri, b_lo, f) for bh pair of this half), then
                    # one strided DVE add pre-sums ri: sqs = re^2 + im^2,
                    # cols (bh, b_lo, f), halving the meas matmul count.
                    key = "sqA" if half == 0 else "sqB"
                    t[key] = sp.tile([DP, 512], dt, name=key, tag=key + g, bufs=2)
                    sq = t[key]
                    nc.scalar.activation(sq[:, 0:512], mm, SQ)
                    ks = key + "s"
                    t[ks] = sp.tile([DP, 256], dt, name=ks, tag=ks + g, bufs=2)
                    APc, VP = type(sq), type(sq.ap)
                    in0 = APc(tensor=sq.tensor, offset=sq.offset,
                              ap=VP([[512, DP], [256, 2], [1, 128]]))
                    in1 = APc(tensor=sq.tensor, offset=sq.offset + 128,
                              ap=VP([[512, DP], [256, 2], [1, 128]]))
                    nc.vector.tensor_tensor(t[ks], in0, in1, ADD)

                def s_meas1():
                    # p-contraction with the OUTPUT TRANSPOSED: the ri-summed
                    # squares are the stationary operand, the per-rep m2
                    # column block the moving one, so out partitions = sq
                    # columns = (bl,f) and out cols = q.  fp16 matmuls run
                    # 1 cyc/row at any width: 4 narrow matmuls replace
                    # meas1 + 4 PE transposes (and their mode-switch drains).
                    mp = new_mm()
                    trm = mp[:, 0:32]
                    mov = c_m2[:, r * 8:(r + 1) * 8]
                    for bh in range(4):
                        sqs = t["sqAs"] if bh < 2 else t["sqBs"]
                        nc.tensor.matmul(trm[:, bh * 8:bh * 8 + 8],
                                         sqs[:, (bh % 2) * 128:(bh % 2) * 128 + 128],
                                         mov, start=True, stop=True)
                    t["trs"] = sp.tile([DP, 32], dt, name="trs", tag="trs" + g,
                                       bufs=2)
                    nc.vector.tensor_copy(t["trs"], trm)

                def s_macc():
                    macc_n[0] += 1
                    nc.tensor.matmul(macc, c_m1[:, r * 24:(r + 1) * 24], t["trs"],
                                     start=(macc_n[0] == 1),
                                     stop=(macc_n[0] == RL))

                st = [s_dma]
                plan = [(None, "P", 0, mv_copy("t1")),
                        ("t1", "F", 0, mv_cmul("s1")),
                        ("s1", "P", 1, mv_copy("t2")),
                        ("t2", "F", 1, mv_cmul("s2")),
                        ("s2", "P", 2, mv_copy("t3")),
                        ("t3", "F", 2, mv_square)]
                for stat_key, side, lidx, move in plan:
                    st.append(mk_half(stat_key, side, lidx, 0, move))
                    st.append(mk_half(stat_key, side, lidx, 1, move))
                st += [s_meas1, s_macc]
                return st

            # three chains cover all 25 reps round-robin, one stage at a
            # time; chain A opens with rep 0 and gets a 4-stage head start
            # (the PE is DMA-bound there anyway), so the lone extra rep
            # rides the DMA ramp instead of draining alone at the tail.
            chains = [
                [emit_rep(rr, ch) for rr in range(ch if ch else 0, RL, 3)]
                for ch in range(3)
            ]
            chains[1] = [emit_rep(rr, 1) for rr in range(1, RL, 3)]
            chains[2] = [emit_rep(rr, 2) for rr in range(2, RL, 3)]
            queues = [[s for rep in chain for s in rep] for chain in chains]
            pos = [0, 0, 0]
            for _ in range(4):
                queues[0][pos[0]]()
                pos[0] += 1
            order = (1, 2, 0)
            while any(pos[i] < len(queues[i]) for i in range(3)):
                for i in order:
                    if pos[i] < len(queues[i]):
                        queues[i][pos[i]]()
                        pos[i] += 1

            # final: copy accumulator to SBUF, DMA out
            outs = sp.tile([24, 32], F32, name="outs", tag="outs")
            nc.vector.tensor_copy(outs, macc)
            nc.sync.dma_start(d_out.ap(), outs)

    nc.compile()
    return nc


# ---------------- public entry ----------------
_CACHE = {}


def _get_nc():
    if "nc" not in _CACHE:
        _CACHE["nc"] = build_nc()
    return _CACHE["nc"]


def run(inputs, trace=False):
    shared, percore = host_prep(inputs["data_angles"], inputs["params"],
                                inputs["noise_choices"])
    nc = _get_nc()
    in_maps = []
    for c in range(NCORES):
        m = dict(shared)
        m.update(percore[c])
        in_maps.append(m)
    res = bass_utils.run_bass_kernel_spmd(nc, in_maps, list(range(NCORES)),
                                          trace=trace)
    acc = np.zeros((24, 32), np.float64)
    for c in range(NCORES):
        acc += np.asarray(res.results[c]["out"], np.float64)
    return decode_output(acc), res


def kernel(**inputs):
    out, _ = run(inputs)
    return out


# revision 24
# speedup vs baseline: 1.3395x; 1.0325x over previous
"""Trainium2 Bass kernel for NoisyPQC (12-qubit noisy PQC expectation values).

Math restructure (validated vs reference in numpy):
  state index s = p*32 + f;  p = qubits 0..6 (qubit 0 = MSB of p),
  f = qubits 7..11.  state.reshape(128,32)[p,f] == state[s].
  Per trajectory r:  x = B3 D B2 D B1 D B0 psi0 with
    B0 = U0, Bl = Ul*Perm(m_{l-1})*Diag(sigma_{l-1}),
    D = (D_P (x) D_F) * C,  C[p,f] = (-1)^((p&1)*((f>>4)&1)).
  D_P/D_F fold into columns of B_l (l>=1); C applied elementwise 3x.
  Layer 0 is trajectory-independent -> host precomputes
    state1 = C * (GP0 @ psi0 @ GF0^T)  for all 16 batches.
  Device per (r): 3 layers of  phi = GP_l @ phi @ GF_l^T  (x C for l<3),
  then probs = |phi|^2, contracted with z-sign tables; final-layer noise
  becomes sign flips folded into the measurement matrices.

v4 scheme ("stationary-state" matmuls as v2, plus):
  - gate tables store [-B|A|B] (1152 cols/rep/tensor): the two moving
    pair planes [A|B] and [-B|A] are overlapping 256-col windows, so the
    HBM gate stream drops 25%.
  - every gate stage is TWO half-stages of 512 psum cols (one bank);
    per-chain mm tag with bufs=2 decouples the PE from the PSUM->SBUF
    moves: the PE runs ahead into the other bank while a move drains.
  - THREE interleaved chains (reps 1..24) + rep 0 solo at the head where
    the PE is DMA-bound anyway.  Each chain's meas1/transpose PSUM lives
    inside its own mm-bank rotation -> 7 banks total, no cross-chain
    PSUM contention.
  - moves: h0 copy on ACT / h1 copy on DVE; xC multiplies on DVE (h0+h1
    via the C table); final-layer moves ARE the ACT squares (PSUM->SBUF
    with Square), no separate square pass, contiguous (not ri-major) -
    meas1 instead sums ri via two strided-moving matmuls per half.

Layouts (b = 4*b_hi + b_lo):
  N: [p=128 part, col = b_hi*256 + ri*128 + b_lo*32 + f]   (ri: 0=Re,1=Im)
  T: [b_lo*32+f = 128 part, col = b_hi*256 + ri*128 + p]
P-stage (N->T), per b_hi: mm pairA=[GPr^T|GPi^T] then pairB=[-GPi^T|GPr^T]
accumulated.  F-stage (T->N) same with kron(I4, .) planes.  All mms
stream 256 cols => full-speed f32r (1 cyc/row).

Sharding: 200 trajectories = 8 cores x 25. Each core accumulates its 25
trajectories' (sign-flipped) measurement vectors into one PSUM bank via a
single open accumulation group; host sums the 8 [24,32] outputs and /200.
"""

import sys

for _p in ("/opt/trn_rl_repo",):
    if _p not in sys.path:
        sys.path.insert(0, _p)

import numpy as np
import ml_dtypes

FP8NP = ml_dtypes.float8_e4m3

from concourse import bacc, bass_utils, mybir
import concourse.tile as tile

# ---------------- problem constants (hardcoded per contract) ----------------
NQ = 12
NL = 4
REPS = 200
BATCH = 16
NCORES = 8
RL = REPS // NCORES  # 25 reps per core
DP, DF = 128, 32  # dim of p-side (qubits 0..6) and f-side (qubits 7..11)
GW = 384  # gate cols per layer: [-B|A|B]; pairB = cols 0:256, pairA = 128:384

F32 = mybir.dt.float32
F32R = mybir.dt.float32r
FP16 = mybir.dt.float16
FP8 = mybir.dt.float8e4


# ---------------- host-side math ----------------
def _rx(t):
    c, s = np.cos(0.5 * t), -1j * np.sin(0.5 * t)
    return np.array([[c, s], [s, c]], np.complex64)


def _rz(t):
    return np.array([[np.exp(-0.5j * t), 0], [0, np.exp(0.5j * t)]], np.complex64)


def _kron_list(mats):
    out = mats[0]
    for m in mats[1:]:
        out = np.kron(out, m)
    return out


def _tables():
    p = np.arange(DP)
    f = np.arange(DF)
    dP = np.ones(DP)
    for j in range(6):
        dP *= np.where(((p >> (6 - j)) & 1) & ((p >> (5 - j)) & 1), -1.0, 1.0)
    dF = np.ones(DF)
    for k in range(4):
        dF *= np.where(((f >> (4 - k)) & 1) & ((f >> (3 - k)) & 1), -1.0, 1.0)
    C = np.where(((p[:, None] & 1) & ((f[None, :] >> 4) & 1)) == 1, -1.0, 1.0)
    zP = 1.0 - 2.0 * ((p[None, :] >> (6 - np.arange(7)[:, None])) & 1)  # [7,128]
    zF = 1.0 - 2.0 * ((f[None, :] >> (4 - np.arange(5)[:, None])) & 1)  # [5,32]
    return dP, dF, C, zP, zF


def host_prep(data_angles, params, noise):
    """Build all device arrays. Returns (shared dict, per-core list of dicts)."""
    da = np.asarray(data_angles, np.float64)
    pa = np.asarray(params, np.float64)
    nz = np.asarray(noise)
    dPt, dFt, C, zP, zF = _tables()

    # base per-qubit gates u[l][q] = Rx(params[l,q,1]) @ Rz(params[l,q,0])
    u = [[_rx(pa[l, q, 1]) @ _rz(pa[l, q, 0]) for q in range(NQ)] for l in range(NL)]

    # --- state after layer 0 (incl. C), identical for every trajectory ---
    va = np.stack([np.cos(0.5 * da), -1j * np.sin(0.5 * da)], -1)  # [B,12,2]
    GP0 = _kron_list([u[0][q] for q in range(7)])        # [128,128]
    GF0 = _kron_list([u[0][q] for q in range(7, NQ)])    # [32,32]
    s_re = np.empty((DP, BATCH * DF), np.float32)
    s_im = np.empty((DP, BATCH * DF), np.float32)
    for b in range(BATCH):
        vPr = _kron_list([va[b, q].astype(np.complex128) for q in range(7)])
        vFr = _kron_list([va[b, q].astype(np.complex128) for q in range(7, NQ)])
        phi = (GP0 @ np.outer(vPr, vFr) @ GF0.T) * C
        s_re[:, b * DF:(b + 1) * DF] = phi.real
        s_im[:, b * DF:(b + 1) * DF] = phi.imag
    # N layout: col = b_hi*256 + ri*128 + b_lo*32 + f
    state1 = np.empty((DP, 1024), np.float32)
    for bh in range(4):
        state1[:, bh * 256:bh * 256 + 128] = s_re[:, bh * 128:bh * 128 + 128]
        state1[:, bh * 256 + 128:bh * 256 + 256] = s_im[:, bh * 128:bh * 128 + 128]

    # --- per-core moving gate tables + measurement tables ---
    # per layer block (GW=384 cols): [-B | A | B] with A = Re-plane, B =
    # Im-plane; pairA=[A|B] = cols 128:384, pairB=[-B|A] = cols 0:256.
    eye4 = np.eye(4)
    percore = []
    for c in range(NCORES):
        gp8 = np.empty((RL, DP, 512), np.float32)   # layer 0: [A|B|-B|A]
        gp16 = np.empty((RL, DP, 2 * GW), np.float32)  # layers 1,2: [-B|A|B]
        gf = np.empty((RL, 32, 9 * DF), np.float32)
        m1 = np.empty((DP, RL * 24), np.float32)
        m2 = np.empty((DP, RL * 8), np.float32)
        for rl in range(RL):
            r = c * RL + rl
            for lidx, l in enumerate((1, 2, 3)):
                facs = []
                for q in range(NQ):
                    g = u[l][q]
                    ch = int(nz[r, l - 1, q])
                    if ch in (1, 2):
                        g = g[:, ::-1]
                    if ch in (2, 3):
                        g = g * np.array([1.0, -1.0])[None, :]
                    facs.append(g)
                GP = _kron_list(facs[:7]) * dPt[None, :]
                GF = _kron_list(facs[7:]) * dFt[None, :]
                if lidx == 0:
                    gp8[rl, :, 0:128] = GP.real.T
                    gp8[rl, :, 128:256] = GP.imag.T
                    gp8[rl, :, 256:384] = -GP.imag.T
                    gp8[rl, :, 384:512] = GP.real.T
                else:
                    o = (lidx - 1) * GW
                    gp16[rl, :, o:o + 128] = -GP.imag.T
                    gp16[rl, :, o + 128:o + 256] = GP.real.T
                    gp16[rl, :, o + 256:o + 384] = GP.imag.T
                o = lidx * GW
                of = lidx * 3 * DF
                gf[rl, :, of:of + DF] = -GF.imag.T
                gf[rl, :, of + DF:of + 2 * DF] = GF.real.T
                gf[rl, :, of + 2 * DF:of + 3 * DF] = GF.imag.T
            m3 = nz[r, NL - 1]  # [12]
            flip = np.where((m3 == 1) | (m3 == 2), -1.0, 1.0)
            M1 = np.ones((DF, 6))
            M1[:, :5] = (zF * flip[7:, None]).T  # [32,5]
            m1[:, rl * 24:(rl + 1) * 24] = np.kron(eye4, M1)
            M2 = np.ones((DP, 8), np.float64)
            M2[:, :7] = (zP * flip[:7, None]).T
            m2[:, rl * 8:(rl + 1) * 8] = M2
        percore.append({"gp8": gp8, "gp16": gp16, "gf": gf, "meas1": m1, "meas2": m2})

    shared = {
        "state1": np.ascontiguousarray(state1),
        "ctabN": np.ascontiguousarray(np.tile(C, (1, 32)).astype(np.float32)),
        "ident": np.eye(8, dtype=np.float32),
    }
    for d in percore:
        for k in list(d):
            d[k] = np.ascontiguousarray(d[k], FP8NP if k == "gp8" else np.float16)
    shared["state1"] = np.ascontiguousarray(shared["state1"], FP8NP)
    shared["ident"] = np.ascontiguousarray(shared["ident"], np.float16)
    return shared, percore


def decode_output(acc):
    """acc: [24,32] summed over cores. Returns [16,12] float32."""
    out = np.empty((BATCH, NQ), np.float64)
    for bh in range(4):
        for bl in range(4):
            b = 4 * bh + bl
            for q in range(7):
                out[b, q] = acc[bl * 6 + 5, bh * 8 + q]
            for j in range(5):
                out[b, 7 + j] = acc[bl * 6 + j, bh * 8 + 7]
    return (out / REPS).astype(np.float32)


# ---------------- device kernel ----------------
def build_nc(dt=FP16, debug=False):
    """Build + compile the per-core Bass program (same for all cores)."""
    nc = bacc.Bacc("TRN2", target_bir_lowering=False, debug=debug,
                   num_devices=NCORES)
    # fp16 everywhere a matmul operand lives: same 1 cyc/row PE rate as
    # f32r but half the DMA/SBUF traffic and far lower PE power (less
    # DVFS throttling).  PSUM stays f32.
    d_state1 = nc.dram_tensor("state1", (DP, 1024), FP8, kind="ExternalInput")
    d_gp8 = nc.dram_tensor("gp8", (RL, DP, 512), FP8, kind="ExternalInput")
    d_gp16 = nc.dram_tensor("gp16", (RL, DP, 2 * GW), dt, kind="ExternalInput")
    d_gf = nc.dram_tensor("gf", (RL, 32, 9 * DF), dt, kind="ExternalInput")
    d_m1 = nc.dram_tensor("meas1", (DP, RL * 24), dt, kind="ExternalInput")
    d_m2 = nc.dram_tensor("meas2", (DP, RL * 8), dt, kind="ExternalInput")
    d_ctn = nc.dram_tensor("ctabN", (DP, 1024), F32, kind="ExternalInput")
    d_id = nc.dram_tensor("ident", (8, 8), dt, kind="ExternalInput")
    d_out = nc.dram_tensor("out", (24, 32), F32, kind="ExternalOutput")

    MUL = mybir.AluOpType.mult
    ADD = mybir.AluOpType.add
    SQ = mybir.ActivationFunctionType.Square

    with tile.TileContext(nc) as tc:
        from contextlib import ExitStack
        with ExitStack() as ex:
            cp = ex.enter_context(tc.tile_pool(name="const", bufs=1))
            sp = ex.enter_context(tc.tile_pool(name="work", bufs=1))
            pp = ex.enter_context(tc.tile_pool(name="ps", bufs=1, space="PSUM"))

            # constants -> SBUF once
            c_state1 = cp.tile([DP, 1024], FP8, name="state1", tag="state1")
            c_ctn = cp.tile([DP, 1024], F32, name="ctn", tag="ctn")
            c_id = cp.tile([8, 8], dt, name="ident", tag="ident")
            c_m1 = cp.tile([DP, RL * 24], dt, name="m1", tag="m1")
            c_m2 = cp.tile([DP, RL * 8], dt, name="m2", tag="m2")
            # consts ride on the ACT queue's DGE ring so the sync+gpsimd
            # rings are free for the per-rep gate streams; need-order.
            nc.scalar.dma_start(c_state1[:, 0:512], d_state1.ap()[:, 0:512])
            nc.scalar.dma_start(c_state1[:, 512:1024], d_state1.ap()[:, 512:1024])
            nc.scalar.dma_start(c_ctn, d_ctn.ap())
            nc.scalar.dma_start(c_m2, d_m2.ap())
            nc.scalar.dma_start(c_m1, d_m1.ap())
            nc.scalar.dma_start(c_id, d_id.ap())

            macc = pp.tile([24, 32], F32, name="macc", tag="macc")
            macc_n = [0]

            # zero the gf double-buffers once: per-rep DMAs only scatter the
            # 4 diagonal kron blocks.  Two dummy allocations per tag walk
            # each rotation exactly one full cycle.
            for zch in range(3):
                for _ in range(2):
                    z = sp.tile([DP, 3 * GW], dt, name="gfz", tag="gf" + str(zch),
                                bufs=2)
                    nc.vector.memset(z.bitcast(mybir.dt.uint32), 0)

            def emit_rep(r, ch):
                """Return list of 16 stage closures for trajectory r, chain ch."""
                t = {}
                g = f"{ch}"

                def s_dma():
                    t["gp8"] = sp.tile([DP, 512], FP8, name="gp8", tag="gp8" + g, bufs=2)
                    t["gp"] = sp.tile([DP, 2 * GW], dt, name="gp", tag="gp" + g, bufs=2)
                    t["gf"] = sp.tile([DP, 3 * GW], dt, name="gf", tag="gf" + g, bufs=2)
                    nc.sync.dma_start(t["gp8"], d_gp8.ap()[r])
                    # gf planes are kron(I4, .): scatter only the 32-row
                    # content into the 4 diagonal blocks; the off-diagonal
                    # zeros were memset once at startup.  Scatters go FIRST:
                    # a rep's gf (F stage, layer 0) is needed three stages
                    # before its gp layer 2.
                    gft = t["gf"]
                    srf = d_gf.ap()[r]
                    APc, VP = type(gft), type(gft.ap)
                    for bl in range(4):
                        dst = APc(tensor=gft.tensor,
                                  offset=gft.offset + bl * 32 * 1152 + bl * 32,
                                  ap=VP([[1152, 32], [128, 9], [1, 32]]))
                        srcb = APc(tensor=srf.tensor, offset=srf.offset,
                                   ap=VP([[288, 32], [32, 9], [1, 32]]))
                        nc.gpsimd.dma_start(dst, srcb)
                    if r <= 6:
                        # head reps: per-layer chunks so the first stages
                        # only wait small transfers while the rings ramp
                        for l3 in range(2):
                            cs = slice(l3 * GW, (l3 + 1) * GW)
                            eng = nc.sync if l3 < 1 else nc.gpsimd
                            eng.dma_start(t["gp"][:, cs], d_gp16.ap()[r][:, cs])
                    else:
                        eng = nc.sync if r % 2 else nc.gpsimd
                        eng.dma_start(t["gp"], d_gp16.ap()[r])

                def new_mm():
                    return pp.tile([DP, 512], F32, name="mm", tag="mm" + g, bufs=2)

                def mk_half(stat_key, side, lidx, half, move):
                    """One half-stage (b_hi pair 2*half, 2*half+1) into a
                    single-bank [128,512] psum tile, then this half's
                    PSUM->SBUF move.  Stationary = state cols of the global
                    b_hi block; moving = the 256-col gate pair windows.
                    Layer-1 P-side runs in fp8 DoubleRow: both re/im K-tiles
                    contract in ONE pass (state1 + gp8 are host-quantized),
                    halving that stage's PE rows."""
                    def s():
                        mm = new_mm()
                        stat = c_state1 if stat_key is None else t[stat_key]
                        if side == "P" and lidx == 0:
                            gp8 = t["gp8"]
                            APc, VP = type(gp8), type(gp8.ap)
                            rhs = APc(tensor=gp8.tensor, offset=gp8.offset,
                                      ap=VP([[512, DP], [256, 2], [1, 256]]))
                            for j, bh in enumerate((2 * half, 2 * half + 1)):
                                o, so = j * 256, bh * 256
                                lhsT = APc(tensor=stat.tensor,
                                           offset=stat.offset + so,
                                           ap=VP([[1024, DP], [128, 2], [1, 128]]))
                                nc.tensor.matmul(
                                    mm[:, o:o + 256], lhsT, rhs,
                                    start=True, stop=True,
                                    perf_mode=mybir.MatmulPerfMode.DoubleRow)
                            move(mm, half)
                            return
                        mov = t["gp"] if side == "P" else t["gf"]
                        base = (lidx - 1) * GW if side == "P" else lidx * GW
                        movA = mov[:, base + 128:base + 384]
                        movB = mov[:, base:base + 256]
                        for j, bh in enumerate((2 * half, 2 * half + 1)):
                            o, so = j * 256, bh * 256
                            nc.tensor.matmul(mm[:, o:o + 256],
                                             stat[:, so:so + 128], movA,
                                             start=True, stop=False)
                            nc.tensor.matmul(mm[:, o:o + 256],
                                             stat[:, so + 128:so + 256], movB,
                                             start=False, stop=True)
                        move(mm, half)
                    return s

                def mv_copy(key):
                    # h0 on ACT, h1 on DVE: each a single 512-col move with a
                    # multi-half-stage window before the bank is needed again.
                    def m(mm, half):
                        if half == 0:
                            t[key] = sp.tile([DP, 1024], dt, name="tmp",
                                             tag="tmp" + g, bufs=2)
                            nc.scalar.copy(t[key][:, 0:512], mm)
                        else:
                            nc.vector.tensor_copy(t[key][:, 512:1024], mm)
                    return m

                def mv_cmul(key):
                    # x C on DVE via the C table (ACT cannot tensor_tensor)
                    def m(mm, half):
                        if half == 0:
                            t[key] = sp.tile([DP, 1024], dt, name="st",
                                             tag="st" + g, bufs=2)
                        cs = slice(half * 512, (half + 1) * 512)
                        nc.vector.tensor_tensor(t[key][:, cs], mm, c_ctn[:, cs], MUL)
                    return m

                def mv_square(mm, half):
                    # final layer: the move IS the square (contiguous layout,
                    # cols (bh, ri, b_lo, f) for bh pair of this half), then
                    # one strided DVE add pre-sums ri: sqs = re^2 + im^2,
                    # cols (bh, b_lo, f), halving the meas matmul count.
                    key = "sqA" if half == 0 else "sqB"
                    t[key] = sp.tile([DP, 512], dt, name=key, tag=key + g, bufs=2)
                    sq = t[key]
                    nc.scalar.activation(sq[:, 0:512], mm, SQ)
                    ks = key + "s"
                    t[ks] = sp.tile([DP, 256], dt, name=ks, tag=ks + g, bufs=2)
                    APc, VP = type(sq), type(sq.ap)
                    in0 = APc(tensor=sq.tensor, offset=sq.offset,
                              ap=VP([[512, DP], [256, 2], [1, 128]]))
                    in1 = APc(tensor=sq.tensor, offset=sq.offset + 128,
                              ap=VP([[512, DP], [256, 2], [1, 128]]))
                    nc.vector.tensor_tensor(t[ks], in0, in1, ADD)

                def s_meas1():
                    # p-contraction with the OUTPUT TRANSPOSED: the ri-summed
                    # squares are the stationary operand, the per-rep m2
                    # column block the moving one, so out partitions = sq
                    # columns = (bl,f) and out cols = q.  fp16 matmuls run
                    # 1 cyc/row at any width: 4 narrow matmuls replace
                    # meas1 + 4 PE transposes (and their mode-switch drains).
                    mp = new_mm()
                    trm = mp[:, 0:32]
                    mov = c_m2[:, r * 8:(r + 1) * 8]
                    for bh in range(4):
                        sqs = t["sqAs"] if bh < 2 else t["sqBs"]
                        nc.tensor.matmul(trm[:, bh * 8:bh * 8 + 8],
                                         sqs[:, (bh % 2) * 128:(bh % 2) * 128 + 128],
                                         mov, start=True, stop=True)
                    t["trs"] = sp.tile([DP, 32], dt, name="trs", tag="trs" + g,
                                       bufs=2)
                    nc.vector.tensor_copy(t["trs"], trm)

                def s_macc():
                    macc_n[0] += 1
                    nc.tensor.matmul(macc, c_m1[:, r * 24:(r + 1) * 24], t["trs"],
                                     start=(macc_n[0] == 1),
                                     stop=(macc_n[0] == RL))

                st = [s_dma]
                plan = [(None, "P", 0, mv_copy("t1")),
                        ("t1", "F", 0, mv_cmul("s1")),
                        ("s1", "P", 1, mv_copy("t2")),
                        ("t2", "F", 1, mv_cmul("s2")),
                        ("s2", "P", 2, mv_copy("t3")),
                        ("t3", "F", 2, mv_square)]
                for stat_key, side, lidx, move in plan:
                    st.append(mk_half(stat_key, side, lidx, 0, move))
                    st.append(mk_half(stat_key, side, lidx, 1, move))
                st += [s_meas1, s_macc]
                return st

            # three chains cover all 25 reps round-robin, one stage at a
            # time; chain A opens with rep 0 and gets a 4-stage head start
            # (the PE is DMA-bound there anyway), so the lone extra rep
            # rides the DMA ramp instead of draining alone at the tail.
            chains = [
                [emit_rep(rr, ch) for rr in range(ch if ch else 0, RL, 3)]
                for ch in range(3)
            ]
            chains[1] = [emit_rep(rr, 1) for rr in range(1, RL, 3)]
            chains[2] = [emit_rep(rr, 2) for rr in range(2, RL, 3)]
            queues = [[s for rep in chain for s in rep] for chain in chains]
            pos = [0, 0, 0]
            for _ in range(4):
                queues[0][pos[0]]()
                pos[0] += 1
            order = (1, 2, 0)
            while any(pos[i] < len(queues[i]) for i in range(3)):
                for i in order:
                    if pos[i] < len(queues[i]):
                        queues[i][pos[i]]()
                        pos[i] += 1

            # final: copy accumulator to SBUF, DMA out
            outs = sp.tile([24, 32], F32, name="outs", tag="outs")
            nc.vector.tensor_copy(outs, macc)
            nc.sync.dma_start(d_out.ap(), outs)

    nc.compile()
    return nc


# ---------------- public entry ----------------
_CACHE = {}


def _get_nc():
    if "nc" not in _CACHE:
        _CACHE["nc"] = build_nc()
    return _CACHE["nc"]


def run(inputs, trace=False):
    shared, percore = host_prep(inputs["data_angles"], inputs["params"],
                                inputs["noise_choices"])
    nc = _get_nc()
    in_maps = []
    for c in range(NCORES):
        m = dict(shared)
        m.update(percore[c])
        in_maps.append(m)
    res = bass_utils.run_bass_kernel_spmd(nc, in_maps, list(range(NCORES)),
                                          trace=trace)
    acc = np.zeros((24, 32), np.float64)
    for c in range(NCORES):
        acc += np.asarray(res.results[c]["out"], np.float64)
    return decode_output(acc), res


def kernel(**inputs):
    out, _ = run(inputs)
    return out
